# revision 20
# baseline (speedup 1.0000x reference)
"""EulerCE attention Trainium2 kernel.

Sharding: data-parallel over batch (2) x head-parallel over 4 head-groups
(16 heads / 4 per group) = 8 cores. Core c: batch c//4, heads 4*(c%4)..+4.

Per-core pipeline (head group g, batch b), all matmul operands bf16
(accumulation f32 in PSUM; rel-err budget 2e-2, measured ~1e-3):

  - QKV projection with host-permuted weight rows so Q/K come out in
    "stacked evens/odds" layout ready for a full-128-partition RoPE-style
    rotation on DVE; V in [n, dh] orientation directly.
  - scores computed transposed: s^T[k, q] = K-slice^T . Q-slice, decay bias
    folded into the exp's per-partition bias (c_h * k is per-partition in
    this layout; the -c_h*q per-row term cancels in softmax). Causal mask
    applied on the PE: a constant accumulate-matmul adds -30000 above the
    diagonal of exact-diagonal 128x128 subtiles, so exp underflows to 0 and
    no vector-engine masking is needed.
  - softmax without max-subtraction (scores provably small for this data),
    denominator obtained by 64 ones-columns in the PV stationary operand
    (PE replicates sum_k P across 64 partitions for free), reciprocal via
    the single-instruction approx-fast DVE op.
  - O-projection consumes attn^T directly; per-core partial outputs are
    summed on host across the 4 head-group cores of each batch.

Scheduling: emission interleaves the QKV projection of chunk s+1 and the
O-projection of strip s-1 into the attention rounds of strip s, so the
tensor engine never idles long enough for the HAM clock gate to drop it
to 1.2 GHz. Scores for tile t are emitted one round ahead of the PV
matmuls of tile t-1 to hide the exp (scalar engine) latency.
"""

import sys

sys.path.insert(0, "/opt/trn_rl_repo")

import math

import numpy as np
import ml_dtypes

import concourse.bass as bass
from concourse import bacc
import concourse.mybir as mybir
import concourse.tile as tile
from concourse.bass_utils import run_bass_kernel_spmd

F32 = mybir.dt.float32
BF16 = mybir.dt.bfloat16
EXP = mybir.ActivationFunctionType.Exp
LN = mybir.ActivationFunctionType.Ln


class _Bacc(bacc.Bacc):
    """Bacc with the activation-table list reordered so the set containing
    both exp and ln is preferred — the default first-match selection picks
    disjoint sets for Exp and Ln and reloads tables (~1.3us + drain) at
    every softmax finalize."""

    def insert_act_table_loads(self):
        import bass_rust as _bass_rust
        from concourse.hw_specs import get_activation_tables
        has_activation = any(
            isinstance(i, mybir.InstActivation)
            for b in self.main_func.blocks
            for i in b.instructions
        )
        if not has_activation:
            return
        tables = list(get_activation_tables(self.m.arch).items())
        # keep list order (set ids may be positional); instead strip exp/ln
        # from every other set so first-match lands on the combined one
        both = [n for n, fns in tables if EXP in fns and LN in fns]
        if both:
            keep = both[0]
            tables = [(n, fns if n == keep else fns - {EXP, LN})
                      for n, fns in tables]
        _bass_rust.insert_act_table_loads(self, tables)

D_MODEL = 1024
N_HEADS = 16
D_HEAD = 64
BATCH = 2
SEQ = 2048
H_LOC = 4          # heads per core
CH = 512           # n-chunk (= strip) size
NCH = SEQ // CH    # 4 chunks
KT = 128           # k tile
NT = SEQ // KT     # 16 n-tiles
NEG = -30000.0     # additive causal mask; exp(x-30000) underflows to 0


def build_program(reps=1, debug=False, hl_merge=True):
    nc = _Bacc()
    xT = nc.dram_tensor("xT", [D_MODEL, SEQ], BF16, kind="ExternalInput")
    wqk = nc.dram_tensor("wqk", [128, 8, 512], BF16, kind="ExternalInput")
    wv = nc.dram_tensor("wv", [128, 8, 256], BF16, kind="ExternalInput")
    wo = nc.dram_tensor("wo", [128, 2, D_MODEL], BF16, kind="ExternalInput")
    cost = nc.dram_tensor("cost", [128, SEQ], F32, kind="ExternalInput")
    sint = nc.dram_tensor("sint", [128, SEQ], F32, kind="ExternalInput")
    biast = nc.dram_tensor("biast", [128, H_LOC * NT], F32, kind="ExternalInput")
    maskt = nc.dram_tensor("maskt", [128, 128], BF16, kind="ExternalInput")
    idnt = nc.dram_tensor("idnt", [128, 128], BF16, kind="ExternalInput")
    out = nc.dram_tensor("out", [SEQ, D_MODEL], BF16, kind="ExternalOutput")

    with tile.TileContext(nc) as tc:
        with (
            tc.tile_pool(name="consts", bufs=1) as consts,
            tc.tile_pool(name="persist", bufs=1) as persist,
            tc.tile_pool(name="xch", bufs=2) as xchp,
            tc.tile_pool(name="rot", bufs=2) as rotp,
            tc.tile_pool(name="ptp", bufs=22) as ptp,
            tc.tile_pool(name="attnp", bufs=4) as attnp,
            tc.tile_pool(name="recp", bufs=2) as recp,
            tc.tile_pool(name="obp", bufs=2) as obp,
            tc.tile_pool(name="qkps", bufs=2, space="PSUM") as qkps,
            tc.tile_pool(name="sps", bufs=2, space="PSUM") as sps,
            tc.tile_pool(name="avps", bufs=1, space="PSUM") as avps,
        ):
            # PE warm-up: ~7us of dependency-free dummy matmuls so the
            # HAM clock gate is released before the first real matmul
            warm_sb = consts.tile([128, CH], BF16, tag="warm")
            nc.vector.memset(warm_sb[:, :], 1.0)
            warm_ps = qkps.tile([128, CH], F32, tag="qkp", name="warm_ps")
            for _ in range(12):
                nc.tensor.matmul(warm_ps[:, :], warm_sb[:, 0:128],
                                 warm_sb[:, :], start=True, stop=True)

            # ---- constants: only wqk before the first x chunk; the rest
            # are emitted later, ordered by first use, so the first QKV
            # matmuls are not stuck behind megabytes of constant DMAs ----
            wqk_sb = consts.tile([128, 8, 512], BF16, tag="wqk")
            nc.sync.dma_start(out=wqk_sb, in_=wqk[:, :, :])
            cos_sb = consts.tile([128, SEQ], F32, tag="cos")
            sin_sb = consts.tile([128, SEQ], F32, tag="sin")
            wv_sb = consts.tile([128, 8, 256], BF16, tag="wv")
            bias_sb = consts.tile([128, H_LOC * NT], F32, tag="bias")
            mask_sb = consts.tile([128, 128], BF16, tag="mask")
            idn_sb = consts.tile([128, 128], BF16, tag="idn")
            wo_sb = consts.tile([128, 2, D_MODEL], BF16, tag="wo")
            actwarm = consts.tile([128, 1], BF16, tag="actwarm")

            def late_consts():
                nc.sync.dma_start(out=cos_sb[:, 0:CH], in_=cost[:, 0:CH])
                nc.sync.dma_start(out=sin_sb[:, 0:CH], in_=sint[:, 0:CH])
                nc.sync.dma_start(out=wv_sb, in_=wv[:, :, :])
                nc.sync.dma_start(out=bias_sb, in_=biast[:, :])
                nc.sync.dma_start(out=mask_sb, in_=maskt[:, :])
                nc.sync.dma_start(out=idn_sb, in_=idnt[:, :])
                nc.sync.dma_start(out=cos_sb[:, CH:], in_=cost[:, CH:])
                nc.sync.dma_start(out=sin_sb[:, CH:], in_=sint[:, CH:])
                nc.sync.dma_start(out=wo_sb, in_=wo[:, :, :])
                # warm the exp table set before the attention phase needs it
                nc.scalar.activation(out=actwarm, in_=bias_sb[:, 0:1], func=EXP,
                                     bias=0.0, scale=0.0)

            # V in [n, dh] layout: [128, ntile, head, 128]; per head block,
            # cols 0:64 = V, cols 64:128 = ones (denominator-replication trick)
            v_sb = persist.tile([128, NT, H_LOC, 128], BF16, tag="vsb")
            nc.vector.memset(v_sb[:, :, :, 64:128], 1.0)

            # packed rotated Q/K, head-pair layout
            qb = [persist.tile([128, SEQ], BF16, tag=f"qb{j}", name=f"qb{j}") for j in range(2)]
            kb = [persist.tile([128, SEQ], BF16, tag=f"kb{j}", name=f"kb{j}") for j in range(2)]

            attn_tiles = {}  # (strip, pair) -> sbuf tile [128, 512] bf16

            def rotate(pe, po, dst, c0):
                # pe/po: psum [128, CH] stacked evens/odds for 4 heads
                # dst: [buf01, buf23]; writes rotated head-pair-packed layout
                t1 = rotp.tile([128, CH], F32, tag="t1")
                t2 = rotp.tile([128, CH], F32, tag="t2")
                t3 = rotp.tile([128, CH], F32, tag="t3")
                t4 = rotp.tile([128, CH], F32, tag="t4")
                top = rotp.tile([128, CH], BF16, tag="top")
                bot = rotp.tile([128, CH], BF16, tag="bot")
                cs = cos_sb[:, c0:c0 + CH]
                sn = sin_sb[:, c0:c0 + CH]
                # both reads of pe first, then both of po, so the PSUM ring
                # slots free as early as possible for the next matmul block
                nc.vector.tensor_mul(t1[:, :], pe[:, :], cs)
                nc.vector.tensor_mul(t3[:, :], pe[:, :], sn)
                nc.vector.tensor_mul(t2[:, :], po[:, :], sn)
                nc.vector.tensor_mul(t4[:, :], po[:, :], cs)
                nc.vector.tensor_sub(top[:, :], t1[:, :], t2[:, :])
                nc.vector.tensor_add(bot[:, :], t3[:, :], t4[:, :])
                # repack: head h (32-row group) -> buf h//2, rows 64*(h%2)+{0:32 top, 32:64 bot}
                for h in range(4):
                    b = dst[h // 2]
                    r0 = 64 * (h % 2)
                    nc.sync.dma_start(out=b[r0:r0 + 32, c0:c0 + CH], in_=top[32 * h:32 * h + 32, :])
                    nc.sync.dma_start(out=b[r0 + 32:r0 + 64, c0:c0 + CH], in_=bot[32 * h:32 * h + 32, :])

            def proj_steps(c):
                # QKV projection of chunk c as a list of emission steps
                c0 = c * CH
                xch = []
                ps = {}

                def load():
                    x = xchp.tile([128, 8, CH], BF16, tag="xch")
                    nc.sync.dma_start(out=x, in_=xT[:, c0:c0 + CH].rearrange("(k p) m -> p k m", p=128))
                    xch.append(x)

                def mkblock(m):
                    def f():
                        p = qkps.tile([128, CH], F32, tag="qkp", name=f"qk_{c}_{m}")
                        for k in range(8):
                            nc.tensor.matmul(
                                p[:, :],
                                wqk_sb[:, k, m * 128:(m + 1) * 128],
                                xch[0][:, k, :],
                                start=(k == 0), stop=(k == 7),
                            )
                        ps[m] = p
                    return f

                def mkrot(m0, m1, dst):
                    def f():
                        rotate(ps[m0], ps[m1], dst, c0)
                    return f

                def mkv(it):
                    def f():
                        t = 4 * c + it
                        vp = qkps.tile([128, CH], F32, tag="qkp", name=f"v_{c}_{it}")
                        for k in range(8):
                            nc.tensor.matmul(
                                vp[:, 0:256],
                                xch[0][:, k, it * 128:(it + 1) * 128],
                                wv_sb[:, k, :],
                                start=(k == 0), stop=(k == 7),
                            )
                        nc.vector.tensor_copy(
                            out=v_sb[:, t, :, 0:64],
                            in_=vp[:, 0:256].rearrange("p (h d) -> p h d", h=4),
                        )
                    return f

                return [load,
                        mkblock(0), mkblock(1), mkrot(0, 1, qb),
                        mkblock(2), mkblock(3), mkrot(2, 3, kb),
                        mkv(0), mkv(1), mkv(2), mkv(3)]

            def sc_act_rounds(s, pr, st):
                # scores + exp for strip s, pair pr: one emission step per
                # k-tile. PV matmuls are NOT emitted here — they are deferred
                # (pt tiles queue in the deep ptp ring) and run as PE filler
                # during the NEXT pair's rounds.
                q0 = s * CH
                ntile = 4 * s + 4
                rounds = []
                for t in range(ntile):
                    def f(t=t):
                        r = t - 4 * s
                        qoff = 128 * r if r >= 0 else 0
                        w = CH - qoff
                        sp = sps.tile([128, 2, CH], F32, tag="sp",
                                      name=f"sp_{s}_{pr}_{t}")
                        for hl in range(2):
                            r0 = 64 * hl
                            nc.tensor.matmul(
                                sp[:, hl, 0:w],
                                kb[pr][r0:r0 + 64, t * KT:(t + 1) * KT],
                                qb[pr][r0:r0 + 64, q0 + qoff:q0 + CH],
                                start=True, stop=(r < 0),
                            )
                            if r >= 0:
                                # additive causal mask on the PE: adds NEG
                                # above the diagonal of the first 128x128
                                # block, exp underflows to 0
                                nc.tensor.matmul(
                                    sp[:, hl, 0:128],
                                    idn_sb[:, :],
                                    mask_sb[:, :],
                                    start=False, stop=True,
                                )
                        pt = ptp.tile([128, 2, CH], BF16, tag="pt",
                                      name=f"pt_{s}_{pr}_{t}")
                        if hl_merge:
                            col = (pr * 2) * NT + t
                            nc.scalar.activation(
                                out=pt[:, :, 0:w], in_=sp[:, :, 0:w], func=EXP,
                                bias=bias_sb[:, col:col + 1], scale=1.0,
                            )
                        else:
                            for hl in range(2):
                                col = (pr * 2 + hl) * NT + t
                                nc.scalar.activation(
                                    out=pt[:, hl, 0:w], in_=sp[:, hl, 0:w],
                                    func=EXP,
                                    bias=bias_sb[:, col:col + 1], scale=1.0,
                                )
                        st["pt"][t] = (pt, w)
                        if debug and (s, pr) == (3, 0) and t == ntile - 1:
                            for hl in range(2):
                                dbg_pt = nc.dram_tensor(
                                    f"dbg_pt{hl}", [128, CH], BF16,
                                    kind="ExternalOutput")
                                nc.sync.dma_start(out=dbg_pt[:, :],
                                                  in_=pt[:, hl, :])
                    rounds.append(f)
                return rounds

            def pv_steps(s, pr, st):
                # deferred PV accumulation + softmax finalize for (s, pr)
                ntile = 4 * s + 4
                steps = []
                for t in range(ntile):
                    def f(t=t):
                        if t == 0:
                            st["avs"] = avps.tile(
                                [128, 2, CH], F32, tag="avs",
                                name=f"avs_{s}_{pr}")
                        pt, w = st["pt"].pop(t)
                        qoff = CH - w
                        for hl in range(2):
                            h = pr * 2 + hl
                            nc.tensor.matmul(
                                st["avs"][:, hl, qoff:CH],
                                v_sb[:, t, h, :],
                                pt[:, hl, 0:w],
                                start=(t == 0), stop=(t == ntile - 1),
                            )
                        if t == ntile - 1:
                            finalize(st, s, pr)
                    steps.append(f)
                return steps

            def finalize(st, s, pr):
                avs = st["avs"]
                if debug and (s, pr) == (3, 0):
                    dbg_avs = nc.dram_tensor("dbg_avs", [128, 2 * CH], F32,
                                             kind="ExternalOutput")
                    avscp = recp.tile([128, 2 * CH], F32, tag="avscp")
                    for hl in range(2):
                        nc.vector.tensor_copy(
                            out=avscp[:, hl * CH:(hl + 1) * CH],
                            in_=avs[:, hl, :])
                    nc.sync.dma_start(out=dbg_avs[:, :], in_=avscp[:, :])
                # rec = 1/den as exp(-ln(den)) on the scalar engine
                # (den >= 1 always; ln+exp share one ACT table set)
                lnd = recp.tile([64, 2 * CH], F32, tag="lnd")
                nc.scalar.activation(
                    out=lnd[:, :],
                    in_=avs[64:128, :, :].rearrange("p a b -> p (a b)"),
                    func=mybir.ActivationFunctionType.Ln,
                )
                rec = recp.tile([64, 2 * CH], F32, tag="rec")
                nc.scalar.activation(
                    out=rec[:, :], in_=lnd[:, :], func=EXP, scale=-1.0)
                at = attnp.tile([128, CH], BF16, tag="attn",
                                name=f"attn_{s}_{pr}")
                attn_tiles[(s, pr)] = at
                for hl in range(2):
                    r0 = 64 * hl
                    nc.vector.tensor_mul(
                        at[r0:r0 + 64, :],
                        avs[0:64, hl, :],
                        rec[:, hl * CH:(hl + 1) * CH],
                    )

            def oproj_steps(s, use_sps=False):
                # O-projection of strip s as 8 emission steps (shares the
                # qkps PSUM ring with the QKV projection; the epilogue also
                # rotates through the then-idle sps ring for pipeline depth)
                steps = []
                for it in range(4):
                    for half in range(2):
                        def f(it=it, half=half):
                            i = 4 * s + it
                            if use_sps and (2 * it + half) % 2 == 1:
                                spt = sps.tile([128, 2, CH], F32, tag="sp",
                                               name=f"op_{s}_{it}_{half}")
                                op = spt[:, 0, :]
                            else:
                                op = qkps.tile([128, CH], F32, tag="qkp",
                                               name=f"op_{s}_{it}_{half}")
                            for ks in range(2):
                                nc.tensor.matmul(
                                    op[:, :],
                                    attn_tiles[(s, ks)][:, it * 128:(it + 1) * 128],
                                    wo_sb[:, ks, half * CH:(half + 1) * CH],
                                    start=(ks == 0), stop=(ks == 1),
                                )
                            ob = obp.tile([128, CH], BF16, tag="ob", name="ob")
                            nc.vector.tensor_copy(out=ob[:, :], in_=op[:, :])
                            nc.sync.dma_start(
                                out=out[i * 128:(i + 1) * 128, half * CH:(half + 1) * CH],
                                in_=ob[:, :],
                            )
                        steps.append(f)
                return steps

            def merge(lists):
                # emit steps from several lists, keeping fractional progress
                # roughly equal (attention rounds pace the phase)
                idx = [0] * len(lists)
                while True:
                    best, bestf = -1, 2.0
                    for i, l in enumerate(lists):
                        if idx[i] < len(l):
                            f = idx[i] / len(l)
                            if f < bestf:
                                best, bestf = i, f
                    if best < 0:
                        break
                    lists[best][idx[best]]()
                    idx[best] += 1

            # ---- schedule ----
            p0 = proj_steps(0)
            p0[0]()          # x chunk 0 DMA right after wqk
            late_consts()
            for step in p0[1:]:
                step()
            pairs = [(s, pr) for s in range(NCH) for pr in range(2)]
            states = {p: {"pt": {}} for p in pairs}
            for k, (s, pr) in enumerate(pairs):
                lists = [sc_act_rounds(s, pr, states[(s, pr)])]
                if k > 0:
                    prev = pairs[k - 1]
                    lists.append(pv_steps(prev[0], prev[1], states[prev]))
                if pr == 0 and s + 1 < NCH:
                    lists.append(proj_steps(s + 1))
                if pr == 1 and s >= 1:
                    lists.append(oproj_steps(s - 1))
                merge(lists)
            for step in pv_steps(NCH - 1, 1, states[(NCH - 1, 1)]):
                step()
            for step in oproj_steps(NCH - 1, use_sps=True):
                step()

            if debug:
                dbg_qb = nc.dram_tensor("dbg_qb", [128, SEQ], BF16, kind="ExternalOutput")
                dbg_kb = nc.dram_tensor("dbg_kb", [128, SEQ], BF16, kind="ExternalOutput")
                dbg_vsb = nc.dram_tensor("dbg_vsb", [128, NT * H_LOC * 128], BF16, kind="ExternalOutput")
                dbg_at = nc.dram_tensor("dbg_at", [128, CH], BF16, kind="ExternalOutput")
                nc.sync.dma_start(out=dbg_qb[:, :], in_=qb[0][:, :])
                nc.sync.dma_start(out=dbg_kb[:, :], in_=kb[0][:, :])
                nc.sync.dma_start(out=dbg_vsb[:, :], in_=v_sb.rearrange("p a b c -> p (a b c)"))
                nc.sync.dma_start(out=dbg_at[:, :], in_=attn_tiles[(3, 0)][:, :])

    return nc


def _sigmoid(v):
    return 1.0 / (1.0 + np.exp(-v.astype(np.float64)))


def build_inputs(x, Wqkv, Wo, log_xi, pi_gate_logit, e_gate_logit):
    x = np.asarray(x, np.float32)
    Wqkv = np.asarray(Wqkv, np.float32)
    Wo = np.asarray(Wo, np.float32)
    log_xi = np.asarray(log_xi, np.float32)
    pi_gate_logit = np.asarray(pi_gate_logit, np.float32)
    e_gate_logit = np.asarray(e_gate_logit, np.float32)

    bf = ml_dtypes.bfloat16
    pi_g = _sigmoid(pi_gate_logit)                      # (16,)
    c_h = (_sigmoid(e_gate_logit) / np.exp(log_xi.astype(np.float64)))  # (16,)

    Wq = Wqkv[0:1024].reshape(N_HEADS, D_HEAD, D_MODEL)
    Wk = Wqkv[1024:2048].reshape(N_HEADS, D_HEAD, D_MODEL)
    Wv = Wqkv[2048:3072].reshape(N_HEADS, D_HEAD, D_MODEL)

    f = np.arange(32)
    inv_freq = np.float64(math.pi) ** (1.0 - 2.0 * f / 64.0)            # (32,)
    pos = np.arange(SEQ, dtype=np.float64)

    # strictly-upper additive causal mask and identity (bf16)
    maskt = np.where(np.arange(128)[:, None] > np.arange(128)[None, :],
                     np.float32(NEG), np.float32(0.0)).astype(bf)
    idnt = np.eye(128, dtype=np.float32).astype(bf)

    in_maps = []
    xTb = [np.ascontiguousarray(x[b].T).astype(bf) for b in range(BATCH)]
    for core in range(8):
        b, g = core // 4, core % 4
        hs = slice(4 * g, 4 * g + 4)
        qe = (Wq[hs, 0::2, :] * 0.125).reshape(128, D_MODEL)
        qo = (Wq[hs, 1::2, :] * 0.125).reshape(128, D_MODEL)
        ke = Wk[hs, 0::2, :].reshape(128, D_MODEL)
        ko = Wk[hs, 1::2, :].reshape(128, D_MODEL)
        # device layout [128 partitions, k, m]: partition p, k-step k holds
        # weight row k*128+p (pre-swizzled so the DMA is contiguous per row)
        wqk = np.ascontiguousarray(
            np.concatenate([qe, qo, ke, ko], 0).T.reshape(8, 128, 512)
            .transpose(1, 0, 2)).astype(bf)
        wv = np.ascontiguousarray(
            Wv[hs].reshape(256, D_MODEL).T.reshape(8, 128, 256)
            .transpose(1, 0, 2)).astype(bf)
        wo = np.ascontiguousarray(
            Wo[:, 256 * g:256 * (g + 1)].T.reshape(2, 128, D_MODEL)
            .transpose(1, 0, 2)).astype(bf)

        theta = pos[None, None, :] * inv_freq[None, :, None] * pi_g[4 * g:4 * g + 4, None, None]
        cost = np.cos(theta).reshape(128, SEQ).astype(np.float32)
        sint = np.sin(theta).reshape(128, SEQ).astype(np.float32)

        biast = np.empty((128, H_LOC * NT), np.float32)
        p = np.arange(128, dtype=np.float64)
        for hl in range(H_LOC):
            for t in range(NT):
                biast[:, hl * NT + t] = (c_h[4 * g + hl] * (128 * t + p)).astype(np.float32)

        in_maps.append({
            "xT": xTb[b], "wqk": wqk, "wv": wv, "wo": wo,
            "cost": cost, "sint": sint, "biast": biast,
            "maskt": maskt, "idnt": idnt,
        })
    return in_maps


def kernel(x, Wqkv, Wo, log_xi, pi_gate_logit, e_gate_logit):
    in_maps = build_inputs(x, Wqkv, Wo, log_xi, pi_gate_logit, e_gate_logit)
    nc = build_program()
    nc.finalize()
    res = run_bass_kernel_spmd(nc, in_maps, list(range(8))).results
    out = np.zeros((BATCH, SEQ, D_MODEL), np.float32)
    for core in range(8):
        out[core // 4] += np.asarray(res[core]["out"]).astype(np.float32)
    return out


# revision 21
# speedup vs baseline: 1.0562x; 1.0562x over previous
"""EulerCE attention Trainium2 kernel.

Sharding: data-parallel over batch (2) x head-parallel over 4 head-groups
(16 heads / 4 per group) = 8 cores. Core c: batch c//4, heads 4*(c%4)..+4.

Per-core pipeline (head group g, batch b), all matmul operands bf16
(accumulation f32 in PSUM; rel-err budget 2e-2, measured ~1e-3):

  - QKV projection with host-permuted weight rows so Q/K come out in
    "stacked evens/odds" layout ready for a full-128-partition RoPE-style
    rotation on DVE; V in [n, dh] orientation directly.
  - scores computed transposed: s^T[k, q] = K-slice^T . Q-slice, decay bias
    folded into the exp's per-partition bias (c_h * k is per-partition in
    this layout; the -c_h*q per-row term cancels in softmax). Causal mask
    applied on the PE: a constant accumulate-matmul adds -30000 above the
    diagonal of exact-diagonal 128x128 subtiles, so exp underflows to 0 and
    no vector-engine masking is needed.
  - softmax without max-subtraction (scores provably small for this data),
    denominator obtained by 64 ones-columns in the PV stationary operand
    (PE replicates sum_k P across 64 partitions for free), reciprocal via
    the single-instruction approx-fast DVE op.
  - O-projection consumes attn^T directly; per-core partial outputs are
    summed on host across the 4 head-group cores of each batch.

Scheduling: emission interleaves the QKV projection of chunk s+1 and the
O-projection of strip s-1 into the attention rounds of strip s, so the
tensor engine never idles long enough for the HAM clock gate to drop it
to 1.2 GHz. Scores for tile t are emitted one round ahead of the PV
matmuls of tile t-1 to hide the exp (scalar engine) latency.
"""

import sys

sys.path.insert(0, "/opt/trn_rl_repo")

import math

import numpy as np
import ml_dtypes

import concourse.bass as bass
from concourse import bacc
import concourse.mybir as mybir
import concourse.tile as tile
from concourse.bass_utils import run_bass_kernel_spmd

F32 = mybir.dt.float32
BF16 = mybir.dt.bfloat16
EXP = mybir.ActivationFunctionType.Exp
LN = mybir.ActivationFunctionType.Ln


class _Bacc(bacc.Bacc):
    """Bacc with the activation-table list reordered so the set containing
    both exp and ln is preferred — the default first-match selection picks
    disjoint sets for Exp and Ln and reloads tables (~1.3us + drain) at
    every softmax finalize."""

    def insert_act_table_loads(self):
        import bass_rust as _bass_rust
        from concourse.hw_specs import get_activation_tables
        has_activation = any(
            isinstance(i, mybir.InstActivation)
            for b in self.main_func.blocks
            for i in b.instructions
        )
        if not has_activation:
            return
        tables = list(get_activation_tables(self.m.arch).items())
        # keep list order (set ids may be positional); instead strip exp/ln
        # from every other set so first-match lands on the combined one
        both = [n for n, fns in tables if EXP in fns and LN in fns]
        if both:
            keep = both[0]
            tables = [(n, fns if n == keep else fns - {EXP, LN})
                      for n, fns in tables]
        _bass_rust.insert_act_table_loads(self, tables)

D_MODEL = 1024
N_HEADS = 16
D_HEAD = 64
BATCH = 2
SEQ = 2048
H_LOC = 4          # heads per core
CH = 512           # n-chunk (= strip) size
NCH = SEQ // CH    # 4 chunks
KT = 128           # k tile
NT = SEQ // KT     # 16 n-tiles
NEG = -30000.0     # additive causal mask; exp(x-30000) underflows to 0


def build_program(reps=1, debug=False, hl_merge=True):
    nc = _Bacc()
    xT = nc.dram_tensor("xT", [D_MODEL, SEQ], BF16, kind="ExternalInput")
    wqk = nc.dram_tensor("wqk", [128, 8, 512], BF16, kind="ExternalInput")
    wv = nc.dram_tensor("wv", [128, 8, 256], BF16, kind="ExternalInput")
    wo = nc.dram_tensor("wo", [128, 2, D_MODEL], BF16, kind="ExternalInput")
    cost = nc.dram_tensor("cost", [128, SEQ], F32, kind="ExternalInput")
    sint = nc.dram_tensor("sint", [128, SEQ], F32, kind="ExternalInput")
    biast = nc.dram_tensor("biast", [128, H_LOC * NT], F32, kind="ExternalInput")
    maskt = nc.dram_tensor("maskt", [128, 128], BF16, kind="ExternalInput")
    idnt = nc.dram_tensor("idnt", [128, 128], BF16, kind="ExternalInput")
    out = nc.dram_tensor("out", [SEQ, D_MODEL], BF16, kind="ExternalOutput")

    with tile.TileContext(nc) as tc:
        with (
            tc.tile_pool(name="consts", bufs=1) as consts,
            tc.tile_pool(name="persist", bufs=1) as persist,
            tc.tile_pool(name="xch", bufs=2) as xchp,
            tc.tile_pool(name="rot", bufs=2) as rotp,
            tc.tile_pool(name="ptp", bufs=22) as ptp,
            tc.tile_pool(name="attnp", bufs=4) as attnp,
            tc.tile_pool(name="recp", bufs=2) as recp,
            tc.tile_pool(name="obp", bufs=2) as obp,
            tc.tile_pool(name="qkps", bufs=2, space="PSUM") as qkps,
            tc.tile_pool(name="sps", bufs=2, space="PSUM") as sps,
            tc.tile_pool(name="avps", bufs=1, space="PSUM") as avps,
        ):
            # PE warm-up: ~7us of dependency-free dummy matmuls so the
            # HAM clock gate is released before the first real matmul
            warm_sb = consts.tile([128, CH], BF16, tag="warm")
            nc.vector.memset(warm_sb[:, :], 1.0)
            warm_ps = qkps.tile([128, CH], F32, tag="qkp", name="warm_ps")
            for _ in range(12):
                nc.tensor.matmul(warm_ps[:, :], warm_sb[:, 0:128],
                                 warm_sb[:, :], start=True, stop=True)

            # ---- constants: only wqk before the first x chunk; the rest
            # are emitted later, ordered by first use, so the first QKV
            # matmuls are not stuck behind megabytes of constant DMAs ----
            wqk_sb = consts.tile([128, 8, 512], BF16, tag="wqk")
            nc.sync.dma_start(out=wqk_sb, in_=wqk[:, :, :])
            cos_sb = consts.tile([128, SEQ], F32, tag="cos")
            sin_sb = consts.tile([128, SEQ], F32, tag="sin")
            wv_sb = consts.tile([128, 8, 256], BF16, tag="wv")
            bias_sb = consts.tile([128, H_LOC * NT], F32, tag="bias")
            mask_sb = consts.tile([128, 128], BF16, tag="mask")
            idn_sb = consts.tile([128, 128], BF16, tag="idn")
            wo_sb = consts.tile([128, 2, D_MODEL], BF16, tag="wo")
            actwarm = consts.tile([128, 1], BF16, tag="actwarm")

            def late_consts():
                nc.sync.dma_start(out=cos_sb[:, 0:CH], in_=cost[:, 0:CH])
                nc.sync.dma_start(out=sin_sb[:, 0:CH], in_=sint[:, 0:CH])
                nc.sync.dma_start(out=wv_sb, in_=wv[:, :, :])
                nc.sync.dma_start(out=bias_sb, in_=biast[:, :])
                nc.sync.dma_start(out=mask_sb, in_=maskt[:, :])
                nc.sync.dma_start(out=idn_sb, in_=idnt[:, :])
                nc.sync.dma_start(out=cos_sb[:, CH:], in_=cost[:, CH:])
                nc.sync.dma_start(out=sin_sb[:, CH:], in_=sint[:, CH:])
                nc.sync.dma_start(out=wo_sb, in_=wo[:, :, :])
                # warm the exp table set before the attention phase needs it
                nc.scalar.activation(out=actwarm, in_=bias_sb[:, 0:1], func=EXP,
                                     bias=0.0, scale=0.0)

            # V in [n, dh] layout: [128, ntile, head, 128]; per head block,
            # cols 0:64 = V, cols 64:128 = ones (denominator-replication trick)
            v_sb = persist.tile([128, NT, H_LOC, 128], BF16, tag="vsb")
            nc.vector.memset(v_sb[:, :, :, 64:128], 1.0)

            # packed rotated Q/K, head-pair layout
            qb = [persist.tile([128, SEQ], BF16, tag=f"qb{j}", name=f"qb{j}") for j in range(2)]
            kb = [persist.tile([128, SEQ], BF16, tag=f"kb{j}", name=f"kb{j}") for j in range(2)]

            attn_tiles = {}  # (strip, pair) -> sbuf tile [128, 512] bf16

            def rotate(pe, po, dst, c0):
                # pe/po: psum [128, CH] stacked evens/odds for 4 heads
                # dst: [buf01, buf23]; writes rotated head-pair-packed layout
                t1 = rotp.tile([128, CH], F32, tag="t1")
                t2 = rotp.tile([128, CH], F32, tag="t2")
                t3 = rotp.tile([128, CH], F32, tag="t3")
                t4 = rotp.tile([128, CH], F32, tag="t4")
                top = rotp.tile([128, CH], BF16, tag="top")
                bot = rotp.tile([128, CH], BF16, tag="bot")
                cs = cos_sb[:, c0:c0 + CH]
                sn = sin_sb[:, c0:c0 + CH]
                # both reads of pe first, then both of po, so the PSUM ring
                # slots free as early as possible for the next matmul block
                nc.vector.tensor_mul(t1[:, :], pe[:, :], cs)
                nc.vector.tensor_mul(t3[:, :], pe[:, :], sn)
                nc.vector.tensor_mul(t2[:, :], po[:, :], sn)
                nc.vector.tensor_mul(t4[:, :], po[:, :], cs)
                nc.vector.tensor_sub(top[:, :], t1[:, :], t2[:, :])
                nc.vector.tensor_add(bot[:, :], t3[:, :], t4[:, :])
                # repack: head h (32-row group) -> buf h//2, rows 64*(h%2)+{0:32 top, 32:64 bot}
                for h in range(4):
                    b = dst[h // 2]
                    r0 = 64 * (h % 2)
                    nc.sync.dma_start(out=b[r0:r0 + 32, c0:c0 + CH], in_=top[32 * h:32 * h + 32, :])
                    nc.sync.dma_start(out=b[r0 + 32:r0 + 64, c0:c0 + CH], in_=bot[32 * h:32 * h + 32, :])

            def proj_steps(c):
                # QKV projection of chunk c as a list of emission steps
                c0 = c * CH
                xch = []
                ps = {}

                def load():
                    x = xchp.tile([128, 8, CH], BF16, tag="xch")
                    nc.sync.dma_start(out=x, in_=xT[:, c0:c0 + CH].rearrange("(k p) m -> p k m", p=128))
                    xch.append(x)

                def mkblock(m):
                    def f():
                        p = qkps.tile([128, CH], F32, tag="qkp", name=f"qk_{c}_{m}")
                        for k in range(8):
                            nc.tensor.matmul(
                                p[:, :],
                                wqk_sb[:, k, m * 128:(m + 1) * 128],
                                xch[0][:, k, :],
                                start=(k == 0), stop=(k == 7),
                            )
                        ps[m] = p
                    return f

                def mkrot(m0, m1, dst):
                    def f():
                        rotate(ps[m0], ps[m1], dst, c0)
                    return f

                def mkv(it):
                    def f():
                        t = 4 * c + it
                        vp = qkps.tile([128, CH], F32, tag="qkp", name=f"v_{c}_{it}")
                        for k in range(8):
                            nc.tensor.matmul(
                                vp[:, 0:256],
                                xch[0][:, k, it * 128:(it + 1) * 128],
                                wv_sb[:, k, :],
                                start=(k == 0), stop=(k == 7),
                            )
                        nc.vector.tensor_copy(
                            out=v_sb[:, t, :, 0:64],
                            in_=vp[:, 0:256].rearrange("p (h d) -> p h d", h=4),
                        )
                    return f

                return ([load, mkblock(0), mkblock(1), mkrot(0, 1, qb)],
                        [mkblock(2), mkblock(3), mkrot(2, 3, kb)],
                        [mkv(0), mkv(1), mkv(2), mkv(3)])

            def sc_act_rounds(s, pr, st):
                # scores + exp for strip s, pair pr: one emission step per
                # k-tile. PV matmuls are NOT emitted here — they are deferred
                # (pt tiles queue in the deep ptp ring) and run as PE filler
                # during the NEXT pair's rounds.
                q0 = s * CH
                ntile = 4 * s + 4
                rounds = []
                for t in range(ntile):
                    def f(t=t):
                        r = t - 4 * s
                        qoff = 128 * r if r >= 0 else 0
                        w = CH - qoff
                        sp = sps.tile([128, 2, CH], F32, tag="sp",
                                      name=f"sp_{s}_{pr}_{t}")
                        for hl in range(2):
                            r0 = 64 * hl
                            nc.tensor.matmul(
                                sp[:, hl, 0:w],
                                kb[pr][r0:r0 + 64, t * KT:(t + 1) * KT],
                                qb[pr][r0:r0 + 64, q0 + qoff:q0 + CH],
                                start=True, stop=(r < 0),
                            )
                            if r >= 0:
                                # additive causal mask on the PE: adds NEG
                                # above the diagonal of the first 128x128
                                # block, exp underflows to 0
                                nc.tensor.matmul(
                                    sp[:, hl, 0:128],
                                    idn_sb[:, :],
                                    mask_sb[:, :],
                                    start=False, stop=True,
                                )
                        pt = ptp.tile([128, 2, CH], BF16, tag="pt",
                                      name=f"pt_{s}_{pr}_{t}")
                        if hl_merge:
                            col = (pr * 2) * NT + t
                            nc.scalar.activation(
                                out=pt[:, :, 0:w], in_=sp[:, :, 0:w], func=EXP,
                                bias=bias_sb[:, col:col + 1], scale=1.0,
                            )
                        else:
                            for hl in range(2):
                                col = (pr * 2 + hl) * NT + t
                                nc.scalar.activation(
                                    out=pt[:, hl, 0:w], in_=sp[:, hl, 0:w],
                                    func=EXP,
                                    bias=bias_sb[:, col:col + 1], scale=1.0,
                                )
                        st["pt"][t] = (pt, w)
                        if debug and (s, pr) == (3, 0) and t == ntile - 1:
                            for hl in range(2):
                                dbg_pt = nc.dram_tensor(
                                    f"dbg_pt{hl}", [128, CH], BF16,
                                    kind="ExternalOutput")
                                nc.sync.dma_start(out=dbg_pt[:, :],
                                                  in_=pt[:, hl, :])
                    rounds.append(f)
                return rounds

            def pv_steps(s, pr, st):
                # deferred PV accumulation + softmax finalize for (s, pr)
                ntile = 4 * s + 4
                steps = []
                for t in range(ntile):
                    def f(t=t):
                        if t == 0:
                            st["avs"] = avps.tile(
                                [128, 2, CH], F32, tag="avs",
                                name=f"avs_{s}_{pr}")
                        pt, w = st["pt"].pop(t)
                        qoff = CH - w
                        for hl in range(2):
                            h = pr * 2 + hl
                            nc.tensor.matmul(
                                st["avs"][:, hl, qoff:CH],
                                v_sb[:, t, h, :],
                                pt[:, hl, 0:w],
                                start=(t == 0), stop=(t == ntile - 1),
                            )
                        if t == ntile - 1:
                            finalize(st, s, pr)
                    steps.append(f)
                return steps

            def finalize(st, s, pr):
                avs = st["avs"]
                if debug and (s, pr) == (3, 0):
                    dbg_avs = nc.dram_tensor("dbg_avs", [128, 2 * CH], F32,
                                             kind="ExternalOutput")
                    avscp = recp.tile([128, 2 * CH], F32, tag="avscp")
                    for hl in range(2):
                        nc.vector.tensor_copy(
                            out=avscp[:, hl * CH:(hl + 1) * CH],
                            in_=avs[:, hl, :])
                    nc.sync.dma_start(out=dbg_avs[:, :], in_=avscp[:, :])
                # rec = 1/den as exp(-ln(den)) on the scalar engine
                # (den >= 1 always; ln+exp share one ACT table set)
                lnd = recp.tile([64, 2 * CH], F32, tag="lnd")
                nc.scalar.activation(
                    out=lnd[:, :],
                    in_=avs[64:128, :, :].rearrange("p a b -> p (a b)"),
                    func=mybir.ActivationFunctionType.Ln,
                )
                rec = recp.tile([64, 2 * CH], F32, tag="rec")
                nc.scalar.activation(
                    out=rec[:, :], in_=lnd[:, :], func=EXP, scale=-1.0)
                at = attnp.tile([128, CH], BF16, tag="attn",
                                name=f"attn_{s}_{pr}")
                attn_tiles[(s, pr)] = at
                for hl in range(2):
                    r0 = 64 * hl
                    nc.vector.tensor_mul(
                        at[r0:r0 + 64, :],
                        avs[0:64, hl, :],
                        rec[:, hl * CH:(hl + 1) * CH],
                    )

            def oproj_steps(s, use_sps=False):
                # O-projection of strip s as 8 emission steps (shares the
                # qkps PSUM ring with the QKV projection; the epilogue also
                # rotates through the then-idle sps ring for pipeline depth)
                steps = []
                for it in range(4):
                    for half in range(2):
                        def f(it=it, half=half):
                            i = 4 * s + it
                            if use_sps and (2 * it + half) % 2 == 1:
                                spt = sps.tile([128, 2, CH], F32, tag="sp",
                                               name=f"op_{s}_{it}_{half}")
                                op = spt[:, 0, :]
                            else:
                                op = qkps.tile([128, CH], F32, tag="qkp",
                                               name=f"op_{s}_{it}_{half}")
                            for ks in range(2):
                                nc.tensor.matmul(
                                    op[:, :],
                                    attn_tiles[(s, ks)][:, it * 128:(it + 1) * 128],
                                    wo_sb[:, ks, half * CH:(half + 1) * CH],
                                    start=(ks == 0), stop=(ks == 1),
                                )
                            ob = obp.tile([128, CH], BF16, tag="ob", name="ob")
                            nc.vector.tensor_copy(out=ob[:, :], in_=op[:, :])
                            nc.sync.dma_start(
                                out=out[i * 128:(i + 1) * 128, half * CH:(half + 1) * CH],
                                in_=ob[:, :],
                            )
                        steps.append(f)
                return steps

            def merge(lists):
                # emit steps from several lists, keeping fractional progress
                # roughly equal (attention rounds pace the phase)
                idx = [0] * len(lists)
                while True:
                    best, bestf = -1, 2.0
                    for i, l in enumerate(lists):
                        if idx[i] < len(l):
                            f = idx[i] / len(l)
                            if f < bestf:
                                best, bestf = i, f
                    if best < 0:
                        break
                    lists[best][idx[best]]()
                    idx[best] += 1

            # ---- schedule ----
            # Each pair (s, pr) is one merge window: its scores+exp rounds,
            # the previous pair's deferred PV matmuls, and PE filler
            # (projection parts / O-projections) placed in the latest window
            # their dependencies allow, to feed the tensor engine through the
            # scalar-heavy late strips.
            q1, k1, v1 = proj_steps(1)
            q2, k2, v2 = proj_steps(2)
            q3, k3, v3 = proj_steps(3)
            op0, op1, op2 = oproj_steps(0), oproj_steps(1), oproj_steps(2)
            pairs = [(s, pr) for s in range(NCH) for pr in range(2)]
            states = {p: {"pt": {}} for p in pairs}
            filler = {
                (0, 0): [q1], (0, 1): [k1],
                (1, 0): [v1, q2], (1, 1): [k2],
                (2, 0): [v2, q3], (2, 1): [op0, op1[:4]],
                (3, 0): [k3, op1[4:]], (3, 1): [v3, op2],
            }
            p0q, p0k, p0v = proj_steps(0)
            p0q[0]()         # x chunk 0 DMA right after wqk
            late_consts()
            for step in p0q[1:] + p0k + p0v:
                step()
            for k, (s, pr) in enumerate(pairs):
                lists = [sc_act_rounds(s, pr, states[(s, pr)])]
                if k > 0:
                    prev = pairs[k - 1]
                    lists.append(pv_steps(prev[0], prev[1], states[prev]))
                lists.extend(filler[(s, pr)])
                merge(lists)
            for step in pv_steps(NCH - 1, 1, states[(NCH - 1, 1)]):
                step()
            for step in oproj_steps(NCH - 1, use_sps=True):
                step()

            if debug:
                dbg_qb = nc.dram_tensor("dbg_qb", [128, SEQ], BF16, kind="ExternalOutput")
                dbg_kb = nc.dram_tensor("dbg_kb", [128, SEQ], BF16, kind="ExternalOutput")
                dbg_vsb = nc.dram_tensor("dbg_vsb", [128, NT * H_LOC * 128], BF16, kind="ExternalOutput")
                dbg_at = nc.dram_tensor("dbg_at", [128, CH], BF16, kind="ExternalOutput")
                nc.sync.dma_start(out=dbg_qb[:, :], in_=qb[0][:, :])
                nc.sync.dma_start(out=dbg_kb[:, :], in_=kb[0][:, :])
                nc.sync.dma_start(out=dbg_vsb[:, :], in_=v_sb.rearrange("p a b c -> p (a b c)"))
                nc.sync.dma_start(out=dbg_at[:, :], in_=attn_tiles[(3, 0)][:, :])

    return nc


def _sigmoid(v):
    return 1.0 / (1.0 + np.exp(-v.astype(np.float64)))


def build_inputs(x, Wqkv, Wo, log_xi, pi_gate_logit, e_gate_logit):
    x = np.asarray(x, np.float32)
    Wqkv = np.asarray(Wqkv, np.float32)
    Wo = np.asarray(Wo, np.float32)
    log_xi = np.asarray(log_xi, np.float32)
    pi_gate_logit = np.asarray(pi_gate_logit, np.float32)
    e_gate_logit = np.asarray(e_gate_logit, np.float32)

    bf = ml_dtypes.bfloat16
    pi_g = _sigmoid(pi_gate_logit)                      # (16,)
    c_h = (_sigmoid(e_gate_logit) / np.exp(log_xi.astype(np.float64)))  # (16,)

    Wq = Wqkv[0:1024].reshape(N_HEADS, D_HEAD, D_MODEL)
    Wk = Wqkv[1024:2048].reshape(N_HEADS, D_HEAD, D_MODEL)
    Wv = Wqkv[2048:3072].reshape(N_HEADS, D_HEAD, D_MODEL)

    f = np.arange(32)
    inv_freq = np.float64(math.pi) ** (1.0 - 2.0 * f / 64.0)            # (32,)
    pos = np.arange(SEQ, dtype=np.float64)

    # strictly-upper additive causal mask and identity (bf16)
    maskt = np.where(np.arange(128)[:, None] > np.arange(128)[None, :],
                     np.float32(NEG), np.float32(0.0)).astype(bf)
    idnt = np.eye(128, dtype=np.float32).astype(bf)

    in_maps = []
    xTb = [np.ascontiguousarray(x[b].T).astype(bf) for b in range(BATCH)]
    for core in range(8):
        b, g = core // 4, core % 4
        hs = slice(4 * g, 4 * g + 4)
        qe = (Wq[hs, 0::2, :] * 0.125).reshape(128, D_MODEL)
        qo = (Wq[hs, 1::2, :] * 0.125).reshape(128, D_MODEL)
        ke = Wk[hs, 0::2, :].reshape(128, D_MODEL)
        ko = Wk[hs, 1::2, :].reshape(128, D_MODEL)
        # device layout [128 partitions, k, m]: partition p, k-step k holds
        # weight row k*128+p (pre-swizzled so the DMA is contiguous per row)
        wqk = np.ascontiguousarray(
            np.concatenate([qe, qo, ke, ko], 0).T.reshape(8, 128, 512)
            .transpose(1, 0, 2)).astype(bf)
        wv = np.ascontiguousarray(
            Wv[hs].reshape(256, D_MODEL).T.reshape(8, 128, 256)
            .transpose(1, 0, 2)).astype(bf)
        wo = np.ascontiguousarray(
            Wo[:, 256 * g:256 * (g + 1)].T.reshape(2, 128, D_MODEL)
            .transpose(1, 0, 2)).astype(bf)

        theta = pos[None, None, :] * inv_freq[None, :, None] * pi_g[4 * g:4 * g + 4, None, None]
        cost = np.cos(theta).reshape(128, SEQ).astype(np.float32)
        sint = np.sin(theta).reshape(128, SEQ).astype(np.float32)

        biast = np.empty((128, H_LOC * NT), np.float32)
        p = np.arange(128, dtype=np.float64)
        for hl in range(H_LOC):
            for t in range(NT):
                biast[:, hl * NT + t] = (c_h[4 * g + hl] * (128 * t + p)).astype(np.float32)

        in_maps.append({
            "xT": xTb[b], "wqk": wqk, "wv": wv, "wo": wo,
            "cost": cost, "sint": sint, "biast": biast,
            "maskt": maskt, "idnt": idnt,
        })
    return in_maps


def kernel(x, Wqkv, Wo, log_xi, pi_gate_logit, e_gate_logit):
    in_maps = build_inputs(x, Wqkv, Wo, log_xi, pi_gate_logit, e_gate_logit)
    nc = build_program()
    nc.finalize()
    res = run_bass_kernel_spmd(nc, in_maps, list(range(8))).results
    out = np.zeros((BATCH, SEQ, D_MODEL), np.float32)
    for core in range(8):
        out[core // 4] += np.asarray(res[core]["out"]).astype(np.float32)
    return out


# revision 22
# speedup vs baseline: 1.1017x; 1.0431x over previous
"""EulerCE attention Trainium2 kernel.

Sharding: data-parallel over batch (2) x head-parallel over 4 head-groups
(16 heads / 4 per group) = 8 cores. Core c: batch c//4, heads 4*(c%4)..+4.

Per-core pipeline (head group g, batch b), all matmul operands bf16
(accumulation f32 in PSUM; rel-err budget 2e-2, measured ~1e-3):

  - QKV projection with host-permuted weight rows so Q/K come out in
    "stacked evens/odds" layout ready for a full-128-partition RoPE-style
    rotation on DVE; V in [n, dh] orientation directly.
  - scores computed transposed: s^T[k, q] = K-slice^T . Q-slice, decay bias
    folded into the exp's per-partition bias (c_h * k is per-partition in
    this layout; the -c_h*q per-row term cancels in softmax). Causal mask
    applied on the PE: a constant accumulate-matmul adds -30000 above the
    diagonal of exact-diagonal 128x128 subtiles, so exp underflows to 0 and
    no vector-engine masking is needed.
  - softmax without max-subtraction (scores provably small for this data),
    denominator obtained by 64 ones-columns in the PV stationary operand
    (PE replicates sum_k P across 64 partitions for free), reciprocal via
    the single-instruction approx-fast DVE op.
  - O-projection consumes attn^T directly; per-core partial outputs are
    summed on host across the 4 head-group cores of each batch.

Scheduling: emission interleaves the QKV projection of chunk s+1 and the
O-projection of strip s-1 into the attention rounds of strip s, so the
tensor engine never idles long enough for the HAM clock gate to drop it
to 1.2 GHz. Scores for tile t are emitted one round ahead of the PV
matmuls of tile t-1 to hide the exp (scalar engine) latency.
"""

import sys

sys.path.insert(0, "/opt/trn_rl_repo")

import math

import numpy as np
import ml_dtypes

import concourse.bass as bass
from concourse import bacc
import concourse.mybir as mybir
import concourse.tile as tile
from concourse.bass_utils import run_bass_kernel_spmd

F32 = mybir.dt.float32
BF16 = mybir.dt.bfloat16
EXP = mybir.ActivationFunctionType.Exp
LN = mybir.ActivationFunctionType.Ln


class _Bacc(bacc.Bacc):
    """Bacc with the activation-table list reordered so the set containing
    both exp and ln is preferred — the default first-match selection picks
    disjoint sets for Exp and Ln and reloads tables (~1.3us + drain) at
    every softmax finalize."""

    def insert_act_table_loads(self):
        import bass_rust as _bass_rust
        from concourse.hw_specs import get_activation_tables
        has_activation = any(
            isinstance(i, mybir.InstActivation)
            for b in self.main_func.blocks
            for i in b.instructions
        )
        if not has_activation:
            return
        tables = list(get_activation_tables(self.m.arch).items())
        # keep list order (set ids may be positional); instead strip exp/ln
        # from every other set so first-match lands on the combined one
        both = [n for n, fns in tables if EXP in fns and LN in fns]
        if both:
            keep = both[0]
            tables = [(n, fns if n == keep else fns - {EXP, LN})
                      for n, fns in tables]
        _bass_rust.insert_act_table_loads(self, tables)

D_MODEL = 1024
N_HEADS = 16
D_HEAD = 64
BATCH = 2
SEQ = 2048
H_LOC = 4          # heads per core
CH = 512           # n-chunk (= strip) size
NCH = SEQ // CH    # 4 chunks
KT = 128           # k tile
NT = SEQ // KT     # 16 n-tiles
NEG = -30000.0     # additive causal mask; exp(x-30000) underflows to 0


def build_program(reps=1, debug=False, hl_merge=True):
    nc = _Bacc()
    xT = nc.dram_tensor("xT", [D_MODEL, SEQ], BF16, kind="ExternalInput")
    wqk = nc.dram_tensor("wqk", [128, 8, 512], BF16, kind="ExternalInput")
    wv = nc.dram_tensor("wv", [128, 8, 256], BF16, kind="ExternalInput")
    wo = nc.dram_tensor("wo", [128, 2, D_MODEL], BF16, kind="ExternalInput")
    cost = nc.dram_tensor("cost", [128, SEQ], F32, kind="ExternalInput")
    sint = nc.dram_tensor("sint", [128, SEQ], F32, kind="ExternalInput")
    biast = nc.dram_tensor("biast", [128, H_LOC * NT], F32, kind="ExternalInput")
    maskt = nc.dram_tensor("maskt", [128, 128], BF16, kind="ExternalInput")
    idnt = nc.dram_tensor("idnt", [128, 128], BF16, kind="ExternalInput")
    out = nc.dram_tensor("out", [SEQ, D_MODEL], BF16, kind="ExternalOutput")

    with tile.TileContext(nc) as tc:
        with (
            tc.tile_pool(name="consts", bufs=1) as consts,
            tc.tile_pool(name="persist", bufs=1) as persist,
            tc.tile_pool(name="xch", bufs=4) as xchp,
            tc.tile_pool(name="rot", bufs=2) as rotp,
            tc.tile_pool(name="ptp", bufs=22) as ptp,
            tc.tile_pool(name="attnp", bufs=4) as attnp,
            tc.tile_pool(name="recp", bufs=2) as recp,
            tc.tile_pool(name="obp", bufs=4) as obp,
            tc.tile_pool(name="qkps", bufs=2, space="PSUM") as qkps,
            tc.tile_pool(name="sps", bufs=2, space="PSUM") as sps,
            tc.tile_pool(name="avps", bufs=1, space="PSUM") as avps,
        ):
            # PE warm-up: ~7us of dependency-free dummy matmuls so the
            # HAM clock gate is released before the first real matmul
            warm_sb = consts.tile([128, CH], BF16, tag="warm")
            nc.vector.memset(warm_sb[:, :], 1.0)
            warm_ps = qkps.tile([128, CH], F32, tag="qkp", name="warm_ps")
            for _ in range(12):
                nc.tensor.matmul(warm_ps[:, :], warm_sb[:, 0:128],
                                 warm_sb[:, :], start=True, stop=True)

            # ---- constants: only wqk before the first x chunk; the rest
            # are emitted later, ordered by first use, so the first QKV
            # matmuls are not stuck behind megabytes of constant DMAs ----
            wqk_sb = consts.tile([128, 8, 512], BF16, tag="wqk")
            nc.sync.dma_start(out=wqk_sb, in_=wqk[:, :, :])
            cos_sb = consts.tile([128, SEQ], F32, tag="cos")
            sin_sb = consts.tile([128, SEQ], F32, tag="sin")
            wv_sb = consts.tile([128, 8, 256], BF16, tag="wv")
            bias_sb = consts.tile([128, H_LOC * NT], F32, tag="bias")
            mask_sb = consts.tile([128, 128], BF16, tag="mask")
            idn_sb = consts.tile([128, 128], BF16, tag="idn")
            wo_sb = consts.tile([128, 2, D_MODEL], BF16, tag="wo")
            actwarm = consts.tile([128, 1], BF16, tag="actwarm")

            def late_consts():
                nc.sync.dma_start(out=cos_sb[:, 0:CH], in_=cost[:, 0:CH])
                nc.sync.dma_start(out=sin_sb[:, 0:CH], in_=sint[:, 0:CH])
                nc.sync.dma_start(out=wv_sb, in_=wv[:, :, :])
                nc.sync.dma_start(out=bias_sb, in_=biast[:, :])
                nc.sync.dma_start(out=mask_sb, in_=maskt[:, :])
                nc.sync.dma_start(out=idn_sb, in_=idnt[:, :])
                nc.sync.dma_start(out=cos_sb[:, CH:], in_=cost[:, CH:])
                nc.sync.dma_start(out=sin_sb[:, CH:], in_=sint[:, CH:])
                nc.sync.dma_start(out=wo_sb, in_=wo[:, :, :])
                # warm the exp table set before the attention phase needs it
                nc.scalar.activation(out=actwarm, in_=bias_sb[:, 0:1], func=EXP,
                                     bias=0.0, scale=0.0)

            # V in [n, dh] layout: [128, ntile, head, 128]; per head block,
            # cols 0:64 = V, cols 64:128 = ones (denominator-replication trick)
            v_sb = persist.tile([128, NT, H_LOC, 128], BF16, tag="vsb")
            nc.vector.memset(v_sb[:, :, :, 64:128], 1.0)

            # packed rotated Q/K, head-pair layout
            qb = [persist.tile([128, SEQ], BF16, tag=f"qb{j}", name=f"qb{j}") for j in range(2)]
            kb = [persist.tile([128, SEQ], BF16, tag=f"kb{j}", name=f"kb{j}") for j in range(2)]

            attn_tiles = {}  # (strip, pair) -> sbuf tile [128, 512] bf16

            def rotate(pe, po, dst, c0):
                # pe/po: psum [128, CH] stacked evens/odds for 4 heads
                # dst: [buf01, buf23]; writes rotated head-pair-packed layout
                t1 = rotp.tile([128, CH], F32, tag="t1")
                t2 = rotp.tile([128, CH], F32, tag="t2")
                t3 = rotp.tile([128, CH], F32, tag="t3")
                t4 = rotp.tile([128, CH], F32, tag="t4")
                top = rotp.tile([128, CH], BF16, tag="top")
                bot = rotp.tile([128, CH], BF16, tag="bot")
                cs = cos_sb[:, c0:c0 + CH]
                sn = sin_sb[:, c0:c0 + CH]
                # both reads of pe first, then both of po, so the PSUM ring
                # slots free as early as possible for the next matmul block
                nc.vector.tensor_mul(t1[:, :], pe[:, :], cs)
                nc.vector.tensor_mul(t3[:, :], pe[:, :], sn)
                nc.vector.tensor_mul(t2[:, :], po[:, :], sn)
                nc.vector.tensor_mul(t4[:, :], po[:, :], cs)
                nc.vector.tensor_sub(top[:, :], t1[:, :], t2[:, :])
                nc.vector.tensor_add(bot[:, :], t3[:, :], t4[:, :])
                # repack: head h (32-row group) -> buf h//2, rows 64*(h%2)+{0:32 top, 32:64 bot}
                for h in range(4):
                    b = dst[h // 2]
                    r0 = 64 * (h % 2)
                    nc.sync.dma_start(out=b[r0:r0 + 32, c0:c0 + CH], in_=top[32 * h:32 * h + 32, :])
                    nc.sync.dma_start(out=b[r0 + 32:r0 + 64, c0:c0 + CH], in_=bot[32 * h:32 * h + 32, :])

            xch_tiles = {}

            def load_chunk(c):
                c0 = c * CH
                x = xchp.tile([128, 8, CH], BF16, tag="xch", name=f"xch{c}")
                nc.sync.dma_start(out=x, in_=xT[:, c0:c0 + CH].rearrange("(k p) m -> p k m", p=128))
                xch_tiles[c] = x

            def proj_steps(c, prologue=False):
                # QKV projection of chunk c as a list of emission steps
                c0 = c * CH
                xch = xch_tiles
                ps = {}

                def mkblock(m):
                    def f():
                        p = qkps.tile([128, CH], F32, tag="qkp", name=f"qk_{c}_{m}")
                        for k in range(8):
                            nc.tensor.matmul(
                                p[:, :],
                                wqk_sb[:, k, m * 128:(m + 1) * 128],
                                xch[c][:, k, :],
                                start=(k == 0), stop=(k == 7),
                            )
                        ps[m] = p
                    return f

                def mkrot(m0, m1, dst):
                    def f():
                        rotate(ps[m0], ps[m1], dst, c0)
                    return f

                def mkv(it):
                    def f():
                        t = 4 * c + it
                        if prologue:
                            vpt = sps.tile([128, 2, CH], F32, tag="sp",
                                           name=f"v_{c}_{it}")
                            vp = vpt[:, 0, :]
                        else:
                            vp = qkps.tile([128, CH], F32, tag="qkp", name=f"v_{c}_{it}")
                        for k in range(8):
                            nc.tensor.matmul(
                                vp[:, 0:256],
                                xch[c][:, k, it * 128:(it + 1) * 128],
                                wv_sb[:, k, :],
                                start=(k == 0), stop=(k == 7),
                            )
                        nc.vector.tensor_copy(
                            out=v_sb[:, t, :, 0:64],
                            in_=vp[:, 0:256].rearrange("p (h d) -> p h d", h=4),
                        )
                    return f

                if prologue:
                    return [mkblock(0), mkblock(1), mkrot(0, 1, qb),
                            mkv(0), mkv(1),
                            mkblock(2), mkblock(3), mkrot(2, 3, kb),
                            mkv(2), mkv(3)]
                return ([mkblock(0), mkblock(1), mkrot(0, 1, qb)],
                        [mkblock(2), mkblock(3), mkrot(2, 3, kb)],
                        [mkv(0), mkv(1), mkv(2), mkv(3)])

            def sc_act_rounds(s, pr, st):
                # scores + exp for strip s, pair pr: one emission step per
                # k-tile. PV matmuls are NOT emitted here — they are deferred
                # (pt tiles queue in the deep ptp ring) and run as PE filler
                # during the NEXT pair's rounds.
                q0 = s * CH
                ntile = 4 * s + 4
                rounds = []
                for t in range(ntile):
                    def f(t=t):
                        r = t - 4 * s
                        qoff = 128 * r if r >= 0 else 0
                        w = CH - qoff
                        sp = sps.tile([128, 2, CH], F32, tag="sp",
                                      name=f"sp_{s}_{pr}_{t}")
                        for hl in range(2):
                            r0 = 64 * hl
                            nc.tensor.matmul(
                                sp[:, hl, 0:w],
                                kb[pr][r0:r0 + 64, t * KT:(t + 1) * KT],
                                qb[pr][r0:r0 + 64, q0 + qoff:q0 + CH],
                                start=True, stop=(r < 0),
                            )
                            if r >= 0:
                                # additive causal mask on the PE: adds NEG
                                # above the diagonal of the first 128x128
                                # block, exp underflows to 0
                                nc.tensor.matmul(
                                    sp[:, hl, 0:128],
                                    idn_sb[:, :],
                                    mask_sb[:, :],
                                    start=False, stop=True,
                                )
                        pt = ptp.tile([128, 2, CH], BF16, tag="pt",
                                      name=f"pt_{s}_{pr}_{t}")
                        if hl_merge:
                            col = (pr * 2) * NT + t
                            nc.scalar.activation(
                                out=pt[:, :, 0:w], in_=sp[:, :, 0:w], func=EXP,
                                bias=bias_sb[:, col:col + 1], scale=1.0,
                            )
                        else:
                            for hl in range(2):
                                col = (pr * 2 + hl) * NT + t
                                nc.scalar.activation(
                                    out=pt[:, hl, 0:w], in_=sp[:, hl, 0:w],
                                    func=EXP,
                                    bias=bias_sb[:, col:col + 1], scale=1.0,
                                )
                        st["pt"][t] = (pt, w)
                        if debug and (s, pr) == (3, 0) and t == ntile - 1:
                            for hl in range(2):
                                dbg_pt = nc.dram_tensor(
                                    f"dbg_pt{hl}", [128, CH], BF16,
                                    kind="ExternalOutput")
                                nc.sync.dma_start(out=dbg_pt[:, :],
                                                  in_=pt[:, hl, :])
                    rounds.append(f)
                return rounds

            def pv_steps(s, pr, st):
                # deferred PV accumulation + softmax finalize for (s, pr)
                ntile = 4 * s + 4
                steps = []
                for t in range(ntile):
                    def f(t=t):
                        if t == 0:
                            st["avs"] = avps.tile(
                                [128, 2, CH], F32, tag="avs",
                                name=f"avs_{s}_{pr}")
                        pt, w = st["pt"].pop(t)
                        qoff = CH - w
                        for hl in range(2):
                            h = pr * 2 + hl
                            nc.tensor.matmul(
                                st["avs"][:, hl, qoff:CH],
                                v_sb[:, t, h, :],
                                pt[:, hl, 0:w],
                                start=(t == 0), stop=(t == ntile - 1),
                            )
                        if t == ntile - 1:
                            finalize(st, s, pr)
                    steps.append(f)
                return steps

            def finalize(st, s, pr):
                avs = st["avs"]
                if debug and (s, pr) == (3, 0):
                    dbg_avs = nc.dram_tensor("dbg_avs", [128, 2 * CH], F32,
                                             kind="ExternalOutput")
                    avscp = recp.tile([128, 2 * CH], F32, tag="avscp")
                    for hl in range(2):
                        nc.vector.tensor_copy(
                            out=avscp[:, hl * CH:(hl + 1) * CH],
                            in_=avs[:, hl, :])
                    nc.sync.dma_start(out=dbg_avs[:, :], in_=avscp[:, :])
                # rec = 1/den as exp(-ln(den)) on the scalar engine
                # (den >= 1 always; ln+exp share one ACT table set)
                lnd = recp.tile([64, 2 * CH], F32, tag="lnd")
                nc.scalar.activation(
                    out=lnd[:, :],
                    in_=avs[64:128, :, :].rearrange("p a b -> p (a b)"),
                    func=mybir.ActivationFunctionType.Ln,
                )
                rec = recp.tile([64, 2 * CH], F32, tag="rec")
                nc.scalar.activation(
                    out=rec[:, :], in_=lnd[:, :], func=EXP, scale=-1.0)
                at = attnp.tile([128, CH], BF16, tag="attn",
                                name=f"attn_{s}_{pr}")
                attn_tiles[(s, pr)] = at
                for hl in range(2):
                    r0 = 64 * hl
                    nc.vector.tensor_mul(
                        at[r0:r0 + 64, :],
                        avs[0:64, hl, :],
                        rec[:, hl * CH:(hl + 1) * CH],
                    )

            def oproj_steps(s, use_sps=False):
                # O-projection of strip s as 8 emission steps (shares the
                # qkps PSUM ring with the QKV projection; the epilogue also
                # rotates through the then-idle sps ring for pipeline depth)
                steps = []
                for it in range(4):
                    for half in range(2):
                        def f(it=it, half=half):
                            i = 4 * s + it
                            if use_sps and (2 * it + half) % 2 == 1:
                                spt = sps.tile([128, 2, CH], F32, tag="sp",
                                               name=f"op_{s}_{it}_{half}")
                                op = spt[:, 0, :]
                            else:
                                op = qkps.tile([128, CH], F32, tag="qkp",
                                               name=f"op_{s}_{it}_{half}")
                            for ks in range(2):
                                nc.tensor.matmul(
                                    op[:, :],
                                    attn_tiles[(s, ks)][:, it * 128:(it + 1) * 128],
                                    wo_sb[:, ks, half * CH:(half + 1) * CH],
                                    start=(ks == 0), stop=(ks == 1),
                                )
                            ob = obp.tile([128, CH], BF16, tag="ob", name="ob")
                            nc.vector.tensor_copy(out=ob[:, :], in_=op[:, :])
                            nc.sync.dma_start(
                                out=out[i * 128:(i + 1) * 128, half * CH:(half + 1) * CH],
                                in_=ob[:, :],
                            )
                        steps.append(f)
                return steps

            def merge(lists):
                # emit steps from several lists, keeping fractional progress
                # roughly equal (attention rounds pace the phase)
                idx = [0] * len(lists)
                while True:
                    best, bestf = -1, 2.0
                    for i, l in enumerate(lists):
                        if idx[i] < len(l):
                            f = idx[i] / len(l)
                            if f < bestf:
                                best, bestf = i, f
                    if best < 0:
                        break
                    lists[best][idx[best]]()
                    idx[best] += 1

            # ---- schedule ----
            # Each pair (s, pr) is one merge window: its scores+exp rounds,
            # the previous pair's deferred PV matmuls, and PE filler
            # (projection parts / O-projections) placed in the latest window
            # their dependencies allow, to feed the tensor engine through the
            # scalar-heavy late strips.
            load_chunk(0)    # x chunk 0 DMA right after wqk
            late_consts()
            for c in range(1, NCH):
                load_chunk(c)
            for step in proj_steps(0, prologue=True):
                step()
            q1, k1, v1 = proj_steps(1)
            q2, k2, v2 = proj_steps(2)
            q3, k3, v3 = proj_steps(3)
            op0, op1, op2 = oproj_steps(0), oproj_steps(1), oproj_steps(2)
            pairs = [(s, pr) for s in range(NCH) for pr in range(2)]
            states = {p: {"pt": {}} for p in pairs}
            filler = {
                (0, 0): [q1], (0, 1): [k1],
                (1, 0): [v1, q2], (1, 1): [k2],
                (2, 0): [v2, q3], (2, 1): [op0, op1[:4]],
                (3, 0): [k3, op1[4:]], (3, 1): [v3, op2],
            }
            for k, (s, pr) in enumerate(pairs):
                lists = [sc_act_rounds(s, pr, states[(s, pr)])]
                if k > 0:
                    prev = pairs[k - 1]
                    lists.append(pv_steps(prev[0], prev[1], states[prev]))
                lists.extend(filler[(s, pr)])
                merge(lists)
            for step in pv_steps(NCH - 1, 1, states[(NCH - 1, 1)]):
                step()
            for step in oproj_steps(NCH - 1, use_sps=True):
                step()

            if debug:
                dbg_qb = nc.dram_tensor("dbg_qb", [128, SEQ], BF16, kind="ExternalOutput")
                dbg_kb = nc.dram_tensor("dbg_kb", [128, SEQ], BF16, kind="ExternalOutput")
                dbg_vsb = nc.dram_tensor("dbg_vsb", [128, NT * H_LOC * 128], BF16, kind="ExternalOutput")
                dbg_at = nc.dram_tensor("dbg_at", [128, CH], BF16, kind="ExternalOutput")
                nc.sync.dma_start(out=dbg_qb[:, :], in_=qb[0][:, :])
                nc.sync.dma_start(out=dbg_kb[:, :], in_=kb[0][:, :])
                nc.sync.dma_start(out=dbg_vsb[:, :], in_=v_sb.rearrange("p a b c -> p (a b c)"))
                nc.sync.dma_start(out=dbg_at[:, :], in_=attn_tiles[(3, 0)][:, :])

    return nc


def _sigmoid(v):
    return 1.0 / (1.0 + np.exp(-v.astype(np.float64)))


def build_inputs(x, Wqkv, Wo, log_xi, pi_gate_logit, e_gate_logit):
    x = np.asarray(x, np.float32)
    Wqkv = np.asarray(Wqkv, np.float32)
    Wo = np.asarray(Wo, np.float32)
    log_xi = np.asarray(log_xi, np.float32)
    pi_gate_logit = np.asarray(pi_gate_logit, np.float32)
    e_gate_logit = np.asarray(e_gate_logit, np.float32)

    bf = ml_dtypes.bfloat16
    pi_g = _sigmoid(pi_gate_logit)                      # (16,)
    c_h = (_sigmoid(e_gate_logit) / np.exp(log_xi.astype(np.float64)))  # (16,)

    Wq = Wqkv[0:1024].reshape(N_HEADS, D_HEAD, D_MODEL)
    Wk = Wqkv[1024:2048].reshape(N_HEADS, D_HEAD, D_MODEL)
    Wv = Wqkv[2048:3072].reshape(N_HEADS, D_HEAD, D_MODEL)

    f = np.arange(32)
    inv_freq = np.float64(math.pi) ** (1.0 - 2.0 * f / 64.0)            # (32,)
    pos = np.arange(SEQ, dtype=np.float64)

    # strictly-upper additive causal mask and identity (bf16)
    maskt = np.where(np.arange(128)[:, None] > np.arange(128)[None, :],
                     np.float32(NEG), np.float32(0.0)).astype(bf)
    idnt = np.eye(128, dtype=np.float32).astype(bf)

    in_maps = []
    xTb = [np.ascontiguousarray(x[b].T).astype(bf) for b in range(BATCH)]
    for core in range(8):
        b, g = core // 4, core % 4
        hs = slice(4 * g, 4 * g + 4)
        qe = (Wq[hs, 0::2, :] * 0.125).reshape(128, D_MODEL)
        qo = (Wq[hs, 1::2, :] * 0.125).reshape(128, D_MODEL)
        ke = Wk[hs, 0::2, :].reshape(128, D_MODEL)
        ko = Wk[hs, 1::2, :].reshape(128, D_MODEL)
        # device layout [128 partitions, k, m]: partition p, k-step k holds
        # weight row k*128+p (pre-swizzled so the DMA is contiguous per row)
        wqk = np.ascontiguousarray(
            np.concatenate([qe, qo, ke, ko], 0).T.reshape(8, 128, 512)
            .transpose(1, 0, 2)).astype(bf)
        wv = np.ascontiguousarray(
            Wv[hs].reshape(256, D_MODEL).T.reshape(8, 128, 256)
            .transpose(1, 0, 2)).astype(bf)
        wo = np.ascontiguousarray(
            Wo[:, 256 * g:256 * (g + 1)].T.reshape(2, 128, D_MODEL)
            .transpose(1, 0, 2)).astype(bf)

        theta = pos[None, None, :] * inv_freq[None, :, None] * pi_g[4 * g:4 * g + 4, None, None]
        cost = np.cos(theta).reshape(128, SEQ).astype(np.float32)
        sint = np.sin(theta).reshape(128, SEQ).astype(np.float32)

        biast = np.empty((128, H_LOC * NT), np.float32)
        p = np.arange(128, dtype=np.float64)
        for hl in range(H_LOC):
            for t in range(NT):
                biast[:, hl * NT + t] = (c_h[4 * g + hl] * (128 * t + p)).astype(np.float32)

        in_maps.append({
            "xT": xTb[b], "wqk": wqk, "wv": wv, "wo": wo,
            "cost": cost, "sint": sint, "biast": biast,
            "maskt": maskt, "idnt": idnt,
        })
    return in_maps


def kernel(x, Wqkv, Wo, log_xi, pi_gate_logit, e_gate_logit):
    in_maps = build_inputs(x, Wqkv, Wo, log_xi, pi_gate_logit, e_gate_logit)
    nc = build_program()
    nc.finalize()
    res = run_bass_kernel_spmd(nc, in_maps, list(range(8))).results
    out = np.zeros((BATCH, SEQ, D_MODEL), np.float32)
    for core in range(8):
        out[core // 4] += np.asarray(res[core]["out"]).astype(np.float32)
    return out


# revision 23
# speedup vs baseline: 1.1688x; 1.0608x over previous
"""EulerCE attention Trainium2 kernel.

Sharding: data-parallel over batch (2) x head-parallel over 4 head-groups
(16 heads / 4 per group) = 8 cores. Core c: batch c//4, heads 4*(c%4)..+4.

Per-core pipeline (head group g, batch b), all matmul operands bf16
(accumulation f32 in PSUM; rel-err budget 2e-2, measured ~1e-3):

  - QKV projection with host-permuted weight rows so Q/K come out in
    "stacked evens/odds" layout ready for a full-128-partition RoPE-style
    rotation on DVE; V in [n, dh] orientation directly.
  - scores computed transposed: s^T[k, q] = K-slice^T . Q-slice, decay bias
    folded into the exp's per-partition bias (c_h * k is per-partition in
    this layout; the -c_h*q per-row term cancels in softmax). Causal mask
    applied on the PE: a constant accumulate-matmul adds -30000 above the
    diagonal of exact-diagonal 128x128 subtiles, so exp underflows to 0 and
    no vector-engine masking is needed.
  - softmax without max-subtraction (scores provably small for this data),
    denominator obtained by 64 ones-columns in the PV stationary operand
    (PE replicates sum_k P across 64 partitions for free), reciprocal via
    the single-instruction approx-fast DVE op.
  - O-projection consumes attn^T directly; per-core partial outputs are
    summed on host across the 4 head-group cores of each batch.

Scheduling: emission interleaves the QKV projection of chunk s+1 and the
O-projection of strip s-1 into the attention rounds of strip s, so the
tensor engine never idles long enough for the HAM clock gate to drop it
to 1.2 GHz. Scores for tile t are emitted one round ahead of the PV
matmuls of tile t-1 to hide the exp (scalar engine) latency.
"""

import sys

sys.path.insert(0, "/opt/trn_rl_repo")

import math

import numpy as np
import ml_dtypes

import concourse.bass as bass
from concourse import bacc
import concourse.mybir as mybir
import concourse.tile as tile
from concourse.bass_utils import run_bass_kernel_spmd

F32 = mybir.dt.float32
BF16 = mybir.dt.bfloat16
EXP = mybir.ActivationFunctionType.Exp
LN = mybir.ActivationFunctionType.Ln


class _Bacc(bacc.Bacc):
    """Bacc with the activation-table list reordered so the set containing
    both exp and ln is preferred — the default first-match selection picks
    disjoint sets for Exp and Ln and reloads tables (~1.3us + drain) at
    every softmax finalize."""

    def insert_act_table_loads(self):
        import bass_rust as _bass_rust
        from concourse.hw_specs import get_activation_tables
        has_activation = any(
            isinstance(i, mybir.InstActivation)
            for b in self.main_func.blocks
            for i in b.instructions
        )
        if not has_activation:
            return
        tables = list(get_activation_tables(self.m.arch).items())
        # keep list order (set ids may be positional); instead strip exp/ln
        # from every other set so first-match lands on the combined one
        both = [n for n, fns in tables if EXP in fns and LN in fns]
        if both:
            keep = both[0]
            tables = [(n, fns if n == keep else fns - {EXP, LN})
                      for n, fns in tables]
        _bass_rust.insert_act_table_loads(self, tables)

D_MODEL = 1024
N_HEADS = 16
D_HEAD = 64
BATCH = 2
SEQ = 2048
H_LOC = 4          # heads per core
CH = 512           # n-chunk (= strip) size
NCH = SEQ // CH    # 4 chunks
KT = 128           # k tile
NT = SEQ // KT     # 16 n-tiles
NEG = -30000.0     # additive causal mask; exp(x-30000) underflows to 0


def build_program(reps=1, debug=False, hl_merge=True):
    nc = _Bacc()
    xT = nc.dram_tensor("xT", [D_MODEL, SEQ], BF16, kind="ExternalInput")
    wqk = nc.dram_tensor("wqk", [128, 8, 512], BF16, kind="ExternalInput")
    wv = nc.dram_tensor("wv", [128, 8, 256], BF16, kind="ExternalInput")
    wo = nc.dram_tensor("wo", [128, 2, D_MODEL], BF16, kind="ExternalInput")
    cost = nc.dram_tensor("cost", [128, SEQ], F32, kind="ExternalInput")
    sint = nc.dram_tensor("sint", [128, SEQ], F32, kind="ExternalInput")
    biast = nc.dram_tensor("biast", [128, H_LOC * NT], F32, kind="ExternalInput")
    maskt = nc.dram_tensor("maskt", [128, 128], BF16, kind="ExternalInput")
    idnt = nc.dram_tensor("idnt", [128, 128], BF16, kind="ExternalInput")
    out = nc.dram_tensor("out", [SEQ, D_MODEL], BF16, kind="ExternalOutput")

    with tile.TileContext(nc) as tc:
        with (
            tc.tile_pool(name="consts", bufs=1) as consts,
            tc.tile_pool(name="persist", bufs=1) as persist,
            tc.tile_pool(name="xch", bufs=4) as xchp,
            tc.tile_pool(name="rot", bufs=2) as rotp,
            tc.tile_pool(name="ptp", bufs=22) as ptp,
            tc.tile_pool(name="attnp", bufs=4) as attnp,
            tc.tile_pool(name="recp", bufs=2) as recp,
            tc.tile_pool(name="obp", bufs=4) as obp,
            tc.tile_pool(name="qkps", bufs=2, space="PSUM") as qkps,
            tc.tile_pool(name="sps", bufs=2, space="PSUM") as sps,
            tc.tile_pool(name="avps", bufs=1, space="PSUM") as avps,
        ):
            # PE warm-up: ~7us of dependency-free dummy matmuls so the
            # HAM clock gate is released before the first real matmul
            warm_sb = consts.tile([128, CH], BF16, tag="warm")
            nc.vector.memset(warm_sb[:, :], 1.0)
            warm_ps = qkps.tile([128, CH], F32, tag="qkp", name="warm_ps")
            for _ in range(12):
                nc.tensor.matmul(warm_ps[:, :], warm_sb[:, 0:128],
                                 warm_sb[:, :], start=True, stop=True)

            # ---- constants: only wqk before the first x chunk; the rest
            # are emitted later, ordered by first use, so the first QKV
            # matmuls are not stuck behind megabytes of constant DMAs ----
            wqk_sb = consts.tile([128, 8, 512], BF16, tag="wqk")
            nc.sync.dma_start(out=wqk_sb, in_=wqk[:, :, :])
            cos_sb = consts.tile([128, SEQ], F32, tag="cos")
            sin_sb = consts.tile([128, SEQ], F32, tag="sin")
            wv_sb = consts.tile([128, 8, 256], BF16, tag="wv")
            bias_sb = consts.tile([128, H_LOC * NT], F32, tag="bias")
            mask_sb = consts.tile([128, 128], BF16, tag="mask")
            idn_sb = consts.tile([128, 128], BF16, tag="idn")
            wo_sb = consts.tile([128, 2, D_MODEL], BF16, tag="wo")
            actwarm = consts.tile([128, 1], BF16, tag="actwarm")

            def late_consts():
                nc.sync.dma_start(out=cos_sb[:, 0:CH], in_=cost[:, 0:CH])
                nc.sync.dma_start(out=sin_sb[:, 0:CH], in_=sint[:, 0:CH])
                nc.sync.dma_start(out=wv_sb, in_=wv[:, :, :])
                nc.sync.dma_start(out=bias_sb, in_=biast[:, :])
                nc.sync.dma_start(out=mask_sb, in_=maskt[:, :])
                nc.sync.dma_start(out=idn_sb, in_=idnt[:, :])
                nc.sync.dma_start(out=cos_sb[:, CH:], in_=cost[:, CH:])
                nc.sync.dma_start(out=sin_sb[:, CH:], in_=sint[:, CH:])
                nc.sync.dma_start(out=wo_sb, in_=wo[:, :, :])
                # warm the exp table set before the attention phase needs it
                nc.scalar.activation(out=actwarm, in_=bias_sb[:, 0:1], func=EXP,
                                     bias=0.0, scale=0.0)

            # V in [n, dh] layout: [128, ntile, head, 128]; per head block,
            # cols 0:64 = V, cols 64:128 = ones (denominator-replication trick)
            v_sb = persist.tile([128, NT, H_LOC, 128], BF16, tag="vsb")
            nc.vector.memset(v_sb[:, :, :, 64:128], 1.0)

            # packed rotated Q/K, head-pair layout
            qb = [persist.tile([128, SEQ], BF16, tag=f"qb{j}", name=f"qb{j}") for j in range(2)]
            kb = [persist.tile([128, SEQ], BF16, tag=f"kb{j}", name=f"kb{j}") for j in range(2)]

            attn_tiles = {}  # (strip, pair) -> sbuf tile [128, 512] bf16

            def rotate(pe, po, dst, c0):
                # pe/po: psum [128, CH] stacked evens/odds for 4 heads
                # dst: [buf01, buf23]; writes rotated head-pair-packed layout
                t1 = rotp.tile([128, CH], F32, tag="t1")
                t2 = rotp.tile([128, CH], F32, tag="t2")
                t3 = rotp.tile([128, CH], F32, tag="t3")
                t4 = rotp.tile([128, CH], F32, tag="t4")
                top = rotp.tile([128, CH], BF16, tag="top")
                bot = rotp.tile([128, CH], BF16, tag="bot")
                cs = cos_sb[:, c0:c0 + CH]
                sn = sin_sb[:, c0:c0 + CH]
                # both reads of pe first, then both of po, so the PSUM ring
                # slots free as early as possible for the next matmul block
                nc.vector.tensor_mul(t1[:, :], pe[:, :], cs)
                nc.vector.tensor_mul(t3[:, :], pe[:, :], sn)
                nc.vector.tensor_mul(t2[:, :], po[:, :], sn)
                nc.vector.tensor_mul(t4[:, :], po[:, :], cs)
                nc.vector.tensor_sub(top[:, :], t1[:, :], t2[:, :])
                nc.vector.tensor_add(bot[:, :], t3[:, :], t4[:, :])
                # repack: head h (32-row group) -> buf h//2, rows 64*(h%2)+{0:32 top, 32:64 bot}
                for h in range(4):
                    b = dst[h // 2]
                    r0 = 64 * (h % 2)
                    nc.sync.dma_start(out=b[r0:r0 + 32, c0:c0 + CH], in_=top[32 * h:32 * h + 32, :])
                    nc.sync.dma_start(out=b[r0 + 32:r0 + 64, c0:c0 + CH], in_=bot[32 * h:32 * h + 32, :])

            xch_tiles = {}

            def load_chunk(c):
                c0 = c * CH
                x = xchp.tile([128, 8, CH], BF16, tag="xch", name=f"xch{c}")
                nc.sync.dma_start(out=x, in_=xT[:, c0:c0 + CH].rearrange("(k p) m -> p k m", p=128))
                xch_tiles[c] = x

            def proj_steps(c, prologue=False):
                # QKV projection of chunk c as a list of emission steps
                c0 = c * CH
                xch = xch_tiles
                ps = {}

                def mkblock(m):
                    def f():
                        p = qkps.tile([128, CH], F32, tag="qkp", name=f"qk_{c}_{m}")
                        for k in range(8):
                            nc.tensor.matmul(
                                p[:, :],
                                wqk_sb[:, k, m * 128:(m + 1) * 128],
                                xch[c][:, k, :],
                                start=(k == 0), stop=(k == 7),
                            )
                        ps[m] = p
                    return f

                def mkrot(m0, m1, dst):
                    def f():
                        rotate(ps[m0], ps[m1], dst, c0)
                    return f

                def mkv(it):
                    def f():
                        t = 4 * c + it
                        if prologue:
                            vpt = sps.tile([128, 2, CH], F32, tag="sp",
                                           name=f"v_{c}_{it}")
                            vp = vpt[:, 0, :]
                        else:
                            vp = qkps.tile([128, CH], F32, tag="qkp", name=f"v_{c}_{it}")
                        for k in range(8):
                            nc.tensor.matmul(
                                vp[:, 0:256],
                                xch[c][:, k, it * 128:(it + 1) * 128],
                                wv_sb[:, k, :],
                                start=(k == 0), stop=(k == 7),
                            )
                        nc.vector.tensor_copy(
                            out=v_sb[:, t, :, 0:64],
                            in_=vp[:, 0:256].rearrange("p (h d) -> p h d", h=4),
                        )
                    return f

                if prologue:
                    return [mkblock(0), mkblock(1), mkrot(0, 1, qb),
                            mkv(0), mkv(1),
                            mkblock(2), mkblock(3), mkrot(2, 3, kb),
                            mkv(2), mkv(3)]
                return ([mkblock(0), mkblock(1), mkrot(0, 1, qb)],
                        [mkblock(2), mkblock(3), mkrot(2, 3, kb)],
                        [mkv(0), mkv(1), mkv(2), mkv(3)])

            def sc_act_rounds(s, pr, st):
                # scores + exp for strip s, pair pr: one emission step per
                # k-tile. PV matmuls are NOT emitted here — they are deferred
                # (pt tiles queue in the deep ptp ring) and run as PE filler
                # during the NEXT pair's rounds.
                q0 = s * CH
                ntile = 4 * s + 4
                rounds = []
                for t in range(ntile):
                    def f(t=t):
                        r = t - 4 * s
                        qoff = 128 * r if r >= 0 else 0
                        w = CH - qoff
                        sp = sps.tile([128, 2, CH], F32, tag="sp",
                                      name=f"sp_{s}_{pr}_{t}")
                        for hl in range(2):
                            r0 = 64 * hl
                            nc.tensor.matmul(
                                sp[:, hl, 0:w],
                                kb[pr][r0:r0 + 64, t * KT:(t + 1) * KT],
                                qb[pr][r0:r0 + 64, q0 + qoff:q0 + CH],
                                start=True, stop=(r < 0),
                            )
                        if r >= 0:
                            # additive causal mask on the PE: adds NEG above
                            # the diagonal of the first 128x128 block, exp
                            # underflows to 0 (adjacent so the idn stationary
                            # loads back-to-back)
                            for hl in range(2):
                                nc.tensor.matmul(
                                    sp[:, hl, 0:128],
                                    idn_sb[:, :],
                                    mask_sb[:, :],
                                    start=False, stop=True,
                                )
                        pt = ptp.tile([128, 2, CH], BF16, tag="pt",
                                      name=f"pt_{s}_{pr}_{t}")
                        if hl_merge:
                            col = (pr * 2) * NT + t
                            nc.scalar.activation(
                                out=pt[:, :, 0:w], in_=sp[:, :, 0:w], func=EXP,
                                bias=bias_sb[:, col:col + 1], scale=1.0,
                            )
                        else:
                            for hl in range(2):
                                col = (pr * 2 + hl) * NT + t
                                nc.scalar.activation(
                                    out=pt[:, hl, 0:w], in_=sp[:, hl, 0:w],
                                    func=EXP,
                                    bias=bias_sb[:, col:col + 1], scale=1.0,
                                )
                        st["pt"][t] = (pt, w)
                        if debug and (s, pr) == (3, 0) and t == ntile - 1:
                            for hl in range(2):
                                dbg_pt = nc.dram_tensor(
                                    f"dbg_pt{hl}", [128, CH], BF16,
                                    kind="ExternalOutput")
                                nc.sync.dma_start(out=dbg_pt[:, :],
                                                  in_=pt[:, hl, :])
                    rounds.append(f)
                return rounds

            def pv_steps(s, pr, st):
                # deferred PV accumulation + softmax finalize for (s, pr)
                ntile = 4 * s + 4
                steps = []
                for t in range(ntile):
                    def f(t=t):
                        if t == 0:
                            st["avs"] = avps.tile(
                                [128, 2, CH], F32, tag="avs",
                                name=f"avs_{s}_{pr}")
                        pt, w = st["pt"].pop(t)
                        qoff = CH - w
                        for hl in range(2):
                            h = pr * 2 + hl
                            nc.tensor.matmul(
                                st["avs"][:, hl, qoff:CH],
                                v_sb[:, t, h, :],
                                pt[:, hl, 0:w],
                                start=(t == 0), stop=(t == ntile - 1),
                            )
                        if t == ntile - 1:
                            finalize(st, s, pr)
                    steps.append(f)
                return steps

            def finalize(st, s, pr):
                avs = st["avs"]
                if debug and (s, pr) == (3, 0):
                    dbg_avs = nc.dram_tensor("dbg_avs", [128, 2 * CH], F32,
                                             kind="ExternalOutput")
                    avscp = recp.tile([128, 2 * CH], F32, tag="avscp")
                    for hl in range(2):
                        nc.vector.tensor_copy(
                            out=avscp[:, hl * CH:(hl + 1) * CH],
                            in_=avs[:, hl, :])
                    nc.sync.dma_start(out=dbg_avs[:, :], in_=avscp[:, :])
                # rec = 1/den as exp(-ln(den)) on the scalar engine
                # (den >= 1 always; ln+exp share one ACT table set)
                lnd = recp.tile([64, 2 * CH], F32, tag="lnd")
                nc.scalar.activation(
                    out=lnd[:, :],
                    in_=avs[64:128, :, :].rearrange("p a b -> p (a b)"),
                    func=mybir.ActivationFunctionType.Ln,
                )
                rec = recp.tile([64, 2 * CH], F32, tag="rec")
                nc.scalar.activation(
                    out=rec[:, :], in_=lnd[:, :], func=EXP, scale=-1.0)
                at = attnp.tile([128, CH], BF16, tag="attn",
                                name=f"attn_{s}_{pr}")
                attn_tiles[(s, pr)] = at
                for hl in range(2):
                    r0 = 64 * hl
                    nc.vector.tensor_mul(
                        at[r0:r0 + 64, :],
                        avs[0:64, hl, :],
                        rec[:, hl * CH:(hl + 1) * CH],
                    )

            def oproj_steps(s, use_sps=False):
                # O-projection of strip s as 8 emission steps (shares the
                # qkps PSUM ring with the QKV projection; the epilogue also
                # rotates through the then-idle sps ring for pipeline depth)
                steps = []
                for it in range(4):
                    for half in range(2):
                        def f(it=it, half=half):
                            i = 4 * s + it
                            if use_sps and (2 * it + half) % 2 == 1:
                                spt = sps.tile([128, 2, CH], F32, tag="sp",
                                               name=f"op_{s}_{it}_{half}")
                                op = spt[:, 0, :]
                            else:
                                op = qkps.tile([128, CH], F32, tag="qkp",
                                               name=f"op_{s}_{it}_{half}")
                            for ks in range(2):
                                nc.tensor.matmul(
                                    op[:, :],
                                    attn_tiles[(s, ks)][:, it * 128:(it + 1) * 128],
                                    wo_sb[:, ks, half * CH:(half + 1) * CH],
                                    start=(ks == 0), stop=(ks == 1),
                                )
                            ob = obp.tile([128, CH], BF16, tag="ob", name="ob")
                            nc.vector.tensor_copy(out=ob[:, :], in_=op[:, :])
                            nc.sync.dma_start(
                                out=out[i * 128:(i + 1) * 128, half * CH:(half + 1) * CH],
                                in_=ob[:, :],
                            )
                        steps.append(f)
                return steps

            def merge(lists):
                # emit steps from several lists, keeping fractional progress
                # roughly equal (attention rounds pace the phase)
                idx = [0] * len(lists)
                while True:
                    best, bestf = -1, 2.0
                    for i, l in enumerate(lists):
                        if idx[i] < len(l):
                            f = idx[i] / len(l)
                            if f < bestf:
                                best, bestf = i, f
                    if best < 0:
                        break
                    lists[best][idx[best]]()
                    idx[best] += 1

            # ---- schedule ----
            # Each pair (s, pr) is one merge window: its scores+exp rounds,
            # the previous pair's deferred PV matmuls, and PE filler
            # (projection parts / O-projections) placed in the latest window
            # their dependencies allow, to feed the tensor engine through the
            # scalar-heavy late strips.
            load_chunk(0)    # x chunk 0 DMA right after wqk
            late_consts()
            for c in range(1, NCH):
                load_chunk(c)
            for step in proj_steps(0, prologue=True):
                step()
            q1, k1, v1 = proj_steps(1)
            q2, k2, v2 = proj_steps(2)
            q3, k3, v3 = proj_steps(3)
            op0, op1, op2 = oproj_steps(0), oproj_steps(1), oproj_steps(2)
            pairs = [(s, pr) for s in range(NCH) for pr in range(2)]
            states = {p: {"pt": {}} for p in pairs}
            filler = {
                (0, 0): [q1], (0, 1): [k1],
                (1, 0): [v1, q2], (1, 1): [k2],
                (2, 0): [v2, q3], (2, 1): [op0, op1[:4]],
                (3, 0): [k3, op1[4:]], (3, 1): [v3, op2],
            }
            for k, (s, pr) in enumerate(pairs):
                # filler first: the in-order PE queue must not stall behind
                # a score matmul that waits on a fresh rotate/repack
                lists = list(filler[(s, pr)])
                if k > 0:
                    prev = pairs[k - 1]
                    lists.append(pv_steps(prev[0], prev[1], states[prev]))
                lists.append(sc_act_rounds(s, pr, states[(s, pr)]))
                merge(lists)
            for step in pv_steps(NCH - 1, 1, states[(NCH - 1, 1)]):
                step()
            for step in oproj_steps(NCH - 1, use_sps=True):
                step()

            if debug:
                dbg_qb = nc.dram_tensor("dbg_qb", [128, SEQ], BF16, kind="ExternalOutput")
                dbg_kb = nc.dram_tensor("dbg_kb", [128, SEQ], BF16, kind="ExternalOutput")
                dbg_vsb = nc.dram_tensor("dbg_vsb", [128, NT * H_LOC * 128], BF16, kind="ExternalOutput")
                dbg_at = nc.dram_tensor("dbg_at", [128, CH], BF16, kind="ExternalOutput")
                nc.sync.dma_start(out=dbg_qb[:, :], in_=qb[0][:, :])
                nc.sync.dma_start(out=dbg_kb[:, :], in_=kb[0][:, :])
                nc.sync.dma_start(out=dbg_vsb[:, :], in_=v_sb.rearrange("p a b c -> p (a b c)"))
                nc.sync.dma_start(out=dbg_at[:, :], in_=attn_tiles[(3, 0)][:, :])

    return nc


def _sigmoid(v):
    return 1.0 / (1.0 + np.exp(-v.astype(np.float64)))


def build_inputs(x, Wqkv, Wo, log_xi, pi_gate_logit, e_gate_logit):
    x = np.asarray(x, np.float32)
    Wqkv = np.asarray(Wqkv, np.float32)
    Wo = np.asarray(Wo, np.float32)
    log_xi = np.asarray(log_xi, np.float32)
    pi_gate_logit = np.asarray(pi_gate_logit, np.float32)
    e_gate_logit = np.asarray(e_gate_logit, np.float32)

    bf = ml_dtypes.bfloat16
    pi_g = _sigmoid(pi_gate_logit)                      # (16,)
    c_h = (_sigmoid(e_gate_logit) / np.exp(log_xi.astype(np.float64)))  # (16,)

    Wq = Wqkv[0:1024].reshape(N_HEADS, D_HEAD, D_MODEL)
    Wk = Wqkv[1024:2048].reshape(N_HEADS, D_HEAD, D_MODEL)
    Wv = Wqkv[2048:3072].reshape(N_HEADS, D_HEAD, D_MODEL)

    f = np.arange(32)
    inv_freq = np.float64(math.pi) ** (1.0 - 2.0 * f / 64.0)            # (32,)
    pos = np.arange(SEQ, dtype=np.float64)

    # strictly-upper additive causal mask and identity (bf16)
    maskt = np.where(np.arange(128)[:, None] > np.arange(128)[None, :],
                     np.float32(NEG), np.float32(0.0)).astype(bf)
    idnt = np.eye(128, dtype=np.float32).astype(bf)

    in_maps = []
    xTb = [np.ascontiguousarray(x[b].T).astype(bf) for b in range(BATCH)]
    for core in range(8):
        b, g = core // 4, core % 4
        hs = slice(4 * g, 4 * g + 4)
        qe = (Wq[hs, 0::2, :] * 0.125).reshape(128, D_MODEL)
        qo = (Wq[hs, 1::2, :] * 0.125).reshape(128, D_MODEL)
        ke = Wk[hs, 0::2, :].reshape(128, D_MODEL)
        ko = Wk[hs, 1::2, :].reshape(128, D_MODEL)
        # device layout [128 partitions, k, m]: partition p, k-step k holds
        # weight row k*128+p (pre-swizzled so the DMA is contiguous per row)
        wqk = np.ascontiguousarray(
            np.concatenate([qe, qo, ke, ko], 0).T.reshape(8, 128, 512)
            .transpose(1, 0, 2)).astype(bf)
        wv = np.ascontiguousarray(
            Wv[hs].reshape(256, D_MODEL).T.reshape(8, 128, 256)
            .transpose(1, 0, 2)).astype(bf)
        wo = np.ascontiguousarray(
            Wo[:, 256 * g:256 * (g + 1)].T.reshape(2, 128, D_MODEL)
            .transpose(1, 0, 2)).astype(bf)

        theta = pos[None, None, :] * inv_freq[None, :, None] * pi_g[4 * g:4 * g + 4, None, None]
        cost = np.cos(theta).reshape(128, SEQ).astype(np.float32)
        sint = np.sin(theta).reshape(128, SEQ).astype(np.float32)

        biast = np.empty((128, H_LOC * NT), np.float32)
        p = np.arange(128, dtype=np.float64)
        for hl in range(H_LOC):
            for t in range(NT):
                biast[:, hl * NT + t] = (c_h[4 * g + hl] * (128 * t + p)).astype(np.float32)

        in_maps.append({
            "xT": xTb[b], "wqk": wqk, "wv": wv, "wo": wo,
            "cost": cost, "sint": sint, "biast": biast,
            "maskt": maskt, "idnt": idnt,
        })
    return in_maps


def kernel(x, Wqkv, Wo, log_xi, pi_gate_logit, e_gate_logit):
    in_maps = build_inputs(x, Wqkv, Wo, log_xi, pi_gate_logit, e_gate_logit)
    nc = build_program()
    nc.finalize()
    res = run_bass_kernel_spmd(nc, in_maps, list(range(8))).results
    out = np.zeros((BATCH, SEQ, D_MODEL), np.float32)
    for core in range(8):
        out[core // 4] += np.asarray(res[core]["out"]).astype(np.float32)
    return out


# revision 27
# speedup vs baseline: 1.1695x; 1.0006x over previous
"""EulerCE attention Trainium2 kernel.

Sharding: data-parallel over batch (2) x head-parallel over 4 head-groups
(16 heads / 4 per group) = 8 cores. Core c: batch c//4, heads 4*(c%4)..+4.

Per-core pipeline (head group g, batch b), all matmul operands bf16
(accumulation f32 in PSUM; rel-err budget 2e-2, measured ~1e-3):

  - QKV projection with host-permuted weight rows so Q/K come out in
    "stacked evens/odds" layout ready for a full-128-partition RoPE-style
    rotation on DVE; V in [n, dh] orientation directly.
  - scores computed transposed: s^T[k, q] = K-slice^T . Q-slice, decay bias
    folded into the exp's per-partition bias (c_h * k is per-partition in
    this layout; the -c_h*q per-row term cancels in softmax). Causal mask
    applied on the PE: a constant accumulate-matmul adds -30000 above the
    diagonal of exact-diagonal 128x128 subtiles, so exp underflows to 0 and
    no vector-engine masking is needed.
  - softmax without max-subtraction (scores provably small for this data),
    denominator obtained by 64 ones-columns in the PV stationary operand
    (PE replicates sum_k P across 64 partitions for free), reciprocal via
    the single-instruction approx-fast DVE op.
  - O-projection consumes attn^T directly; per-core partial outputs are
    summed on host across the 4 head-group cores of each batch.

Scheduling: emission interleaves the QKV projection of chunk s+1 and the
O-projection of strip s-1 into the attention rounds of strip s, so the
tensor engine never idles long enough for the HAM clock gate to drop it
to 1.2 GHz. Scores for tile t are emitted one round ahead of the PV
matmuls of tile t-1 to hide the exp (scalar engine) latency.
"""

import sys

sys.path.insert(0, "/opt/trn_rl_repo")

import math

import numpy as np
import ml_dtypes

import concourse.bass as bass
from concourse import bacc
import concourse.mybir as mybir
import concourse.tile as tile
from concourse.bass_utils import run_bass_kernel_spmd

F32 = mybir.dt.float32
BF16 = mybir.dt.bfloat16
EXP = mybir.ActivationFunctionType.Exp
LN = mybir.ActivationFunctionType.Ln


class _Bacc(bacc.Bacc):
    """Bacc with the activation-table list reordered so the set containing
    both exp and ln is preferred — the default first-match selection picks
    disjoint sets for Exp and Ln and reloads tables (~1.3us + drain) at
    every softmax finalize."""

    def insert_act_table_loads(self):
        import bass_rust as _bass_rust
        from concourse.hw_specs import get_activation_tables
        has_activation = any(
            isinstance(i, mybir.InstActivation)
            for b in self.main_func.blocks
            for i in b.instructions
        )
        if not has_activation:
            return
        tables = list(get_activation_tables(self.m.arch).items())
        # keep list order (set ids may be positional); instead strip exp/ln
        # from every other set so first-match lands on the combined one
        both = [n for n, fns in tables if EXP in fns and LN in fns]
        if both:
            keep = both[0]
            tables = [(n, fns if n == keep else fns - {EXP, LN})
                      for n, fns in tables]
        _bass_rust.insert_act_table_loads(self, tables)

D_MODEL = 1024
N_HEADS = 16
D_HEAD = 64
BATCH = 2
SEQ = 2048
H_LOC = 4          # heads per core
CH = 512           # n-chunk (= strip) size
NCH = SEQ // CH    # 4 chunks
KT = 128           # k tile
NT = SEQ // KT     # 16 n-tiles
NEG = -30000.0     # additive causal mask; exp(x-30000) underflows to 0


def build_program(reps=1, debug=False, hl_merge=True):
    nc = _Bacc()
    xT = nc.dram_tensor("xT", [D_MODEL, SEQ], BF16, kind="ExternalInput")
    wqk = nc.dram_tensor("wqk", [128, 8, 512], BF16, kind="ExternalInput")
    wv = nc.dram_tensor("wv", [128, 8, 256], BF16, kind="ExternalInput")
    wo = nc.dram_tensor("wo", [128, 2, D_MODEL], BF16, kind="ExternalInput")
    cost = nc.dram_tensor("cost", [128, SEQ], F32, kind="ExternalInput")
    sint = nc.dram_tensor("sint", [128, SEQ], F32, kind="ExternalInput")
    biast = nc.dram_tensor("biast", [128, H_LOC * NT], F32, kind="ExternalInput")
    maskt = nc.dram_tensor("maskt", [128, 128], BF16, kind="ExternalInput")
    idnt = nc.dram_tensor("idnt", [128, 128], BF16, kind="ExternalInput")
    out = nc.dram_tensor("out", [SEQ, D_MODEL], BF16, kind="ExternalOutput")

    with tile.TileContext(nc) as tc:
        with (
            tc.tile_pool(name="consts", bufs=1) as consts,
            tc.tile_pool(name="persist", bufs=1) as persist,
            tc.tile_pool(name="xch", bufs=4) as xchp,
            tc.tile_pool(name="rot", bufs=2) as rotp,
            tc.tile_pool(name="ptp", bufs=22) as ptp,
            tc.tile_pool(name="attnp", bufs=4) as attnp,
            tc.tile_pool(name="recp", bufs=2) as recp,
            tc.tile_pool(name="obp", bufs=4) as obp,
            tc.tile_pool(name="qkps", bufs=2, space="PSUM") as qkps,
            tc.tile_pool(name="sps", bufs=2, space="PSUM") as sps,
            tc.tile_pool(name="avps", bufs=1, space="PSUM") as avps,
        ):
            # PE warm-up: ~7us of dependency-free dummy matmuls so the
            # HAM clock gate is released before the first real matmul
            warm_sb = consts.tile([128, CH], BF16, tag="warm")
            nc.vector.memset(warm_sb[:, :], 1.0)
            warm_ps = qkps.tile([128, CH], F32, tag="qkp", name="warm_ps")
            for _ in range(12):
                nc.tensor.matmul(warm_ps[:, :], warm_sb[:, 0:128],
                                 warm_sb[:, :], start=True, stop=True)

            # ---- constants: only wqk before the first x chunk; the rest
            # are emitted later, ordered by first use, so the first QKV
            # matmuls are not stuck behind megabytes of constant DMAs ----
            wqk_sb = consts.tile([128, 8, 512], BF16, tag="wqk")
            nc.sync.dma_start(out=wqk_sb, in_=wqk[:, :, :])
            cos_sb = consts.tile([128, SEQ], F32, tag="cos")
            sin_sb = consts.tile([128, SEQ], F32, tag="sin")
            wv_sb = consts.tile([128, 8, 256], BF16, tag="wv")
            bias_sb = consts.tile([128, H_LOC * NT], F32, tag="bias")
            mask_sb = consts.tile([128, 128], BF16, tag="mask")
            idn_sb = consts.tile([128, 128], BF16, tag="idn")
            wo_sb = consts.tile([128, 2, D_MODEL], BF16, tag="wo")
            actwarm = consts.tile([128, 1], BF16, tag="actwarm")

            def late_consts():
                nc.sync.dma_start(out=cos_sb[:, 0:CH], in_=cost[:, 0:CH])
                nc.sync.dma_start(out=sin_sb[:, 0:CH], in_=sint[:, 0:CH])
                nc.sync.dma_start(out=wv_sb, in_=wv[:, :, :])
                nc.sync.dma_start(out=bias_sb, in_=biast[:, :])
                nc.sync.dma_start(out=mask_sb, in_=maskt[:, :])
                nc.sync.dma_start(out=idn_sb, in_=idnt[:, :])
                nc.sync.dma_start(out=cos_sb[:, CH:], in_=cost[:, CH:])
                nc.sync.dma_start(out=sin_sb[:, CH:], in_=sint[:, CH:])
                nc.sync.dma_start(out=wo_sb, in_=wo[:, :, :])
                # warm the exp table set before the attention phase needs it
                nc.scalar.activation(out=actwarm, in_=bias_sb[:, 0:1], func=EXP,
                                     bias=0.0, scale=0.0)

            # V in [n, dh] layout: [128, ntile, head, 128]; per head block,
            # cols 0:64 = V, cols 64:128 = ones (denominator-replication trick)
            v_sb = persist.tile([128, NT, H_LOC, 128], BF16, tag="vsb")
            nc.vector.memset(v_sb[:, :, :, 64:128], 1.0)

            # packed rotated Q/K, head-pair layout
            qb = [persist.tile([128, SEQ], BF16, tag=f"qb{j}", name=f"qb{j}") for j in range(2)]
            kb = [persist.tile([128, SEQ], BF16, tag=f"kb{j}", name=f"kb{j}") for j in range(2)]

            attn_tiles = {}  # (strip, pair) -> sbuf tile [128, 512] bf16

            def rotate(pe, po, dst, c0):
                # pe/po: psum [128, CH] stacked evens/odds for 4 heads
                # dst: [buf01, buf23]; writes rotated head-pair-packed layout
                t1 = rotp.tile([128, CH], F32, tag="t1")
                t2 = rotp.tile([128, CH], F32, tag="t2")
                t3 = rotp.tile([128, CH], F32, tag="t3")
                t4 = rotp.tile([128, CH], F32, tag="t4")
                top = rotp.tile([128, CH], BF16, tag="top")
                bot = rotp.tile([128, CH], BF16, tag="bot")
                cs = cos_sb[:, c0:c0 + CH]
                sn = sin_sb[:, c0:c0 + CH]
                # both reads of pe first, then both of po, so the PSUM ring
                # slots free as early as possible for the next matmul block
                nc.vector.tensor_mul(t1[:, :], pe[:, :], cs)
                nc.vector.tensor_mul(t3[:, :], pe[:, :], sn)
                nc.vector.tensor_mul(t2[:, :], po[:, :], sn)
                nc.vector.tensor_mul(t4[:, :], po[:, :], cs)
                nc.vector.tensor_sub(top[:, :], t1[:, :], t2[:, :])
                nc.vector.tensor_add(bot[:, :], t3[:, :], t4[:, :])
                # repack: head h (32-row group) -> buf h//2, rows 64*(h%2)+{0:32 top, 32:64 bot}
                for h in range(4):
                    b = dst[h // 2]
                    r0 = 64 * (h % 2)
                    nc.sync.dma_start(out=b[r0:r0 + 32, c0:c0 + CH], in_=top[32 * h:32 * h + 32, :])
                    nc.sync.dma_start(out=b[r0 + 32:r0 + 64, c0:c0 + CH], in_=bot[32 * h:32 * h + 32, :])

            xch_tiles = {}

            def load_chunk(c):
                c0 = c * CH
                x = xchp.tile([128, 8, CH], BF16, tag="xch", name=f"xch{c}")
                nc.sync.dma_start(out=x, in_=xT[:, c0:c0 + CH].rearrange("(k p) m -> p k m", p=128))
                xch_tiles[c] = x

            def proj_steps(c, prologue=False):
                # QKV projection of chunk c as a list of emission steps
                c0 = c * CH
                xch = xch_tiles
                ps = {}

                def mkblock(m):
                    def f():
                        p = qkps.tile([128, CH], F32, tag="qkp", name=f"qk_{c}_{m}")
                        for k in range(8):
                            nc.tensor.matmul(
                                p[:, :],
                                wqk_sb[:, k, m * 128:(m + 1) * 128],
                                xch[c][:, k, :],
                                start=(k == 0), stop=(k == 7),
                            )
                        ps[m] = p
                    return f

                def mkrot(m0, m1, dst):
                    def f():
                        rotate(ps[m0], ps[m1], dst, c0)
                    return f

                def mkv(it):
                    def f():
                        t = 4 * c + it
                        if prologue:
                            vpt = sps.tile([128, 2, CH], F32, tag="sp",
                                           name=f"v_{c}_{it}")
                            vp = vpt[:, 0, :]
                        else:
                            vp = qkps.tile([128, CH], F32, tag="qkp", name=f"v_{c}_{it}")
                        for k in range(8):
                            nc.tensor.matmul(
                                vp[:, 0:256],
                                xch[c][:, k, it * 128:(it + 1) * 128],
                                wv_sb[:, k, :],
                                start=(k == 0), stop=(k == 7),
                            )
                        nc.vector.tensor_copy(
                            out=v_sb[:, t, :, 0:64],
                            in_=vp[:, 0:256].rearrange("p (h d) -> p h d", h=4),
                        )
                    return f

                if prologue:
                    return [mkblock(0), mkblock(1), mkrot(0, 1, qb),
                            mkv(0), mkv(1),
                            mkblock(2), mkblock(3), mkrot(2, 3, kb),
                            mkv(2), mkv(3)]
                return ([mkblock(0), mkblock(1), mkrot(0, 1, qb)],
                        [mkblock(2), mkblock(3), mkrot(2, 3, kb)],
                        [mkv(0), mkv(1), mkv(2), mkv(3)])

            def sc_act_rounds(s, pr, st):
                # scores + exp for strip s, pair pr: one emission step per
                # k-tile. PV matmuls are NOT emitted here — they are deferred
                # (pt tiles queue in the deep ptp ring) and run as PE filler
                # during the NEXT pair's rounds.
                q0 = s * CH
                ntile = 4 * s + 4
                rounds = []
                for t in range(ntile):
                    def f(t=t):
                        r = t - 4 * s
                        qoff = 128 * r if r >= 0 else 0
                        w = CH - qoff
                        sp = sps.tile([128, 2, CH], F32, tag="sp",
                                      name=f"sp_{s}_{pr}_{t}")
                        for hl in range(2):
                            r0 = 64 * hl
                            nc.tensor.matmul(
                                sp[:, hl, 0:w],
                                kb[pr][r0:r0 + 64, t * KT:(t + 1) * KT],
                                qb[pr][r0:r0 + 64, q0 + qoff:q0 + CH],
                                start=True, stop=(r < 0),
                            )
                        if r >= 0:
                            # additive causal mask on the PE: adds NEG above
                            # the diagonal of the first 128x128 block, exp
                            # underflows to 0 (adjacent so the idn stationary
                            # loads back-to-back)
                            for hl in range(2):
                                nc.tensor.matmul(
                                    sp[:, hl, 0:128],
                                    idn_sb[:, :],
                                    mask_sb[:, :],
                                    start=False, stop=True,
                                )
                        pt = ptp.tile([128, 2, CH], BF16, tag="pt",
                                      name=f"pt_{s}_{pr}_{t}")
                        if hl_merge:
                            col = (pr * 2) * NT + t
                            nc.scalar.activation(
                                out=pt[:, :, 0:w], in_=sp[:, :, 0:w], func=EXP,
                                bias=bias_sb[:, col:col + 1], scale=1.0,
                            )
                        else:
                            for hl in range(2):
                                col = (pr * 2 + hl) * NT + t
                                nc.scalar.activation(
                                    out=pt[:, hl, 0:w], in_=sp[:, hl, 0:w],
                                    func=EXP,
                                    bias=bias_sb[:, col:col + 1], scale=1.0,
                                )
                        st["pt"][t] = (pt, w)
                        if debug and (s, pr) == (3, 0) and t == ntile - 1:
                            for hl in range(2):
                                dbg_pt = nc.dram_tensor(
                                    f"dbg_pt{hl}", [128, CH], BF16,
                                    kind="ExternalOutput")
                                nc.sync.dma_start(out=dbg_pt[:, :],
                                                  in_=pt[:, hl, :])
                    rounds.append(f)
                return rounds

            def pv_steps(s, pr, st):
                # deferred PV accumulation + softmax finalize for (s, pr)
                ntile = 4 * s + 4
                steps = []
                for t in range(ntile):
                    def f(t=t):
                        if t == 0:
                            st["avs"] = avps.tile(
                                [128, 2, CH], F32, tag="avs",
                                name=f"avs_{s}_{pr}")
                        pt, w = st["pt"].pop(t)
                        qoff = CH - w
                        for hl in range(2):
                            h = pr * 2 + hl
                            nc.tensor.matmul(
                                st["avs"][:, hl, qoff:CH],
                                v_sb[:, t, h, :],
                                pt[:, hl, 0:w],
                                start=(t == 0), stop=(t == ntile - 1),
                            )
                        if t == ntile - 1:
                            finalize(st, s, pr)
                    steps.append(f)
                return steps

            def finalize(st, s, pr):
                avs = st["avs"]
                if debug and (s, pr) == (3, 0):
                    dbg_avs = nc.dram_tensor("dbg_avs", [128, 2 * CH], F32,
                                             kind="ExternalOutput")
                    avscp = recp.tile([128, 2 * CH], F32, tag="avscp")
                    for hl in range(2):
                        nc.vector.tensor_copy(
                            out=avscp[:, hl * CH:(hl + 1) * CH],
                            in_=avs[:, hl, :])
                    nc.sync.dma_start(out=dbg_avs[:, :], in_=avscp[:, :])
                # rec = 1/den as exp(-ln(den)) on the scalar engine
                # (den >= 1 always; ln+exp share one ACT table set)
                lnd = recp.tile([64, 2 * CH], F32, tag="lnd")
                nc.scalar.activation(
                    out=lnd[:, :],
                    in_=avs[64:128, :, :].rearrange("p a b -> p (a b)"),
                    func=mybir.ActivationFunctionType.Ln,
                )
                rec = recp.tile([64, 2 * CH], F32, tag="rec")
                nc.scalar.activation(
                    out=rec[:, :], in_=lnd[:, :], func=EXP, scale=-1.0)
                at = attnp.tile([128, CH], BF16, tag="attn",
                                name=f"attn_{s}_{pr}")
                attn_tiles[(s, pr)] = at
                for hl in range(2):
                    r0 = 64 * hl
                    nc.vector.tensor_mul(
                        at[r0:r0 + 64, :],
                        avs[0:64, hl, :],
                        rec[:, hl * CH:(hl + 1) * CH],
                    )

            def oproj_steps(s, use_sps=False):
                # O-projection of strip s as 8 emission steps (shares the
                # qkps PSUM ring with the QKV projection; the epilogue also
                # rotates through the then-idle sps ring for pipeline depth)
                steps = []
                for it in range(4):
                    for half in range(2):
                        def f(it=it, half=half):
                            i = 4 * s + it
                            if use_sps and (2 * it + half) % 2 == 1:
                                spt = sps.tile([128, 2, CH], F32, tag="sp",
                                               name=f"op_{s}_{it}_{half}")
                                op = spt[:, 0, :]
                            else:
                                op = qkps.tile([128, CH], F32, tag="qkp",
                                               name=f"op_{s}_{it}_{half}")
                            for ks in range(2):
                                nc.tensor.matmul(
                                    op[:, :],
                                    attn_tiles[(s, ks)][:, it * 128:(it + 1) * 128],
                                    wo_sb[:, ks, half * CH:(half + 1) * CH],
                                    start=(ks == 0), stop=(ks == 1),
                                )
                            ob = obp.tile([128, CH], BF16, tag="ob", name="ob")
                            nc.vector.tensor_copy(out=ob[:, :], in_=op[:, :])
                            nc.sync.dma_start(
                                out=out[i * 128:(i + 1) * 128, half * CH:(half + 1) * CH],
                                in_=ob[:, :],
                            )
                        steps.append(f)
                return steps

            def merge(lists):
                # emit steps from several lists, keeping fractional progress
                # roughly equal; a (steps, weight) entry with weight w
                # finishes when the others are at 1/w of their length
                norm = [l if isinstance(l, tuple) else (l, 1.0) for l in lists]
                idx = [0] * len(norm)
                while True:
                    best, bestf = -1, None
                    for i, (l, wt) in enumerate(norm):
                        if idx[i] < len(l):
                            f = idx[i] / (len(l) * wt)
                            if bestf is None or f < bestf:
                                best, bestf = i, f
                    if best < 0:
                        break
                    norm[best][0][idx[best]]()
                    idx[best] += 1

            # ---- schedule ----
            # Each pair (s, pr) is one merge window: its scores+exp rounds,
            # the previous pair's deferred PV matmuls, and PE filler
            # (projection parts / O-projections) placed in the latest window
            # their dependencies allow, to feed the tensor engine through the
            # scalar-heavy late strips.
            load_chunk(0)    # x chunk 0 DMA right after wqk
            late_consts()
            for c in range(1, NCH):
                load_chunk(c)
            for step in proj_steps(0, prologue=True):
                step()
            q1, k1, v1 = proj_steps(1)
            q2, k2, v2 = proj_steps(2)
            q3, k3, v3 = proj_steps(3)
            op0, op1, op2 = oproj_steps(0), oproj_steps(1), oproj_steps(2)
            pairs = [(s, pr) for s in range(NCH) for pr in range(2)]
            states = {p: {"pt": {}} for p in pairs}
            filler = {
                (0, 0): [(q1, 2.0)], (0, 1): [(k1, 2.0)],
                (1, 0): [(v1, 2.0), (q2, 1.5)], (1, 1): [(k2, 2.0)],
                (2, 0): [(v2, 2.0), (q3, 1.5)], (2, 1): [op0, op1[:4]],
                (3, 0): [(k3, 2.0), op1[4:]], (3, 1): [(v3, 2.0), op2[:5]],
            }
            for k, (s, pr) in enumerate(pairs):
                # filler first: the in-order PE queue must not stall behind
                # a score matmul that waits on a fresh rotate/repack
                lists = list(filler[(s, pr)])
                if k > 0:
                    prev = pairs[k - 1]
                    lists.append(pv_steps(prev[0], prev[1], states[prev]))
                lists.append(sc_act_rounds(s, pr, states[(s, pr)]))
                merge(lists)
            # epilogue: last pair's PVs merged with leftover O-projection
            merge([
                pv_steps(NCH - 1, 1, states[(NCH - 1, 1)]),
                op2[5:],
            ])
            for step in oproj_steps(NCH - 1, use_sps=True):
                step()

            if debug:
                dbg_qb = nc.dram_tensor("dbg_qb", [128, SEQ], BF16, kind="ExternalOutput")
                dbg_kb = nc.dram_tensor("dbg_kb", [128, SEQ], BF16, kind="ExternalOutput")
                dbg_vsb = nc.dram_tensor("dbg_vsb", [128, NT * H_LOC * 128], BF16, kind="ExternalOutput")
                dbg_at = nc.dram_tensor("dbg_at", [128, CH], BF16, kind="ExternalOutput")
                nc.sync.dma_start(out=dbg_qb[:, :], in_=qb[0][:, :])
                nc.sync.dma_start(out=dbg_kb[:, :], in_=kb[0][:, :])
                nc.sync.dma_start(out=dbg_vsb[:, :], in_=v_sb.rearrange("p a b c -> p (a b c)"))
                nc.sync.dma_start(out=dbg_at[:, :], in_=attn_tiles[(3, 0)][:, :])

    return nc


def _sigmoid(v):
    return 1.0 / (1.0 + np.exp(-v.astype(np.float64)))


def build_inputs(x, Wqkv, Wo, log_xi, pi_gate_logit, e_gate_logit):
    x = np.asarray(x, np.float32)
    Wqkv = np.asarray(Wqkv, np.float32)
    Wo = np.asarray(Wo, np.float32)
    log_xi = np.asarray(log_xi, np.float32)
    pi_gate_logit = np.asarray(pi_gate_logit, np.float32)
    e_gate_logit = np.asarray(e_gate_logit, np.float32)

    bf = ml_dtypes.bfloat16
    pi_g = _sigmoid(pi_gate_logit)                      # (16,)
    c_h = (_sigmoid(e_gate_logit) / np.exp(log_xi.astype(np.float64)))  # (16,)

    Wq = Wqkv[0:1024].reshape(N_HEADS, D_HEAD, D_MODEL)
    Wk = Wqkv[1024:2048].reshape(N_HEADS, D_HEAD, D_MODEL)
    Wv = Wqkv[2048:3072].reshape(N_HEADS, D_HEAD, D_MODEL)

    f = np.arange(32)
    inv_freq = np.float64(math.pi) ** (1.0 - 2.0 * f / 64.0)            # (32,)
    pos = np.arange(SEQ, dtype=np.float64)

    # strictly-upper additive causal mask and identity (bf16)
    maskt = np.where(np.arange(128)[:, None] > np.arange(128)[None, :],
                     np.float32(NEG), np.float32(0.0)).astype(bf)
    idnt = np.eye(128, dtype=np.float32).astype(bf)

    in_maps = []
    xTb = [np.ascontiguousarray(x[b].T).astype(bf) for b in range(BATCH)]
    for core in range(8):
        b, g = core // 4, core % 4
        hs = slice(4 * g, 4 * g + 4)
        qe = (Wq[hs, 0::2, :] * 0.125).reshape(128, D_MODEL)
        qo = (Wq[hs, 1::2, :] * 0.125).reshape(128, D_MODEL)
        ke = Wk[hs, 0::2, :].reshape(128, D_MODEL)
        ko = Wk[hs, 1::2, :].reshape(128, D_MODEL)
        # device layout [128 partitions, k, m]: partition p, k-step k holds
        # weight row k*128+p (pre-swizzled so the DMA is contiguous per row)
        wqk = np.ascontiguousarray(
            np.concatenate([qe, qo, ke, ko], 0).T.reshape(8, 128, 512)
            .transpose(1, 0, 2)).astype(bf)
        wv = np.ascontiguousarray(
            Wv[hs].reshape(256, D_MODEL).T.reshape(8, 128, 256)
            .transpose(1, 0, 2)).astype(bf)
        wo = np.ascontiguousarray(
            Wo[:, 256 * g:256 * (g + 1)].T.reshape(2, 128, D_MODEL)
            .transpose(1, 0, 2)).astype(bf)

        theta = pos[None, None, :] * inv_freq[None, :, None] * pi_g[4 * g:4 * g + 4, None, None]
        cost = np.cos(theta).reshape(128, SEQ).astype(np.float32)
        sint = np.sin(theta).reshape(128, SEQ).astype(np.float32)

        biast = np.empty((128, H_LOC * NT), np.float32)
        p = np.arange(128, dtype=np.float64)
        for hl in range(H_LOC):
            for t in range(NT):
                biast[:, hl * NT + t] = (c_h[4 * g + hl] * (128 * t + p)).astype(np.float32)

        in_maps.append({
            "xT": xTb[b], "wqk": wqk, "wv": wv, "wo": wo,
            "cost": cost, "sint": sint, "biast": biast,
            "maskt": maskt, "idnt": idnt,
        })
    return in_maps


def kernel(x, Wqkv, Wo, log_xi, pi_gate_logit, e_gate_logit):
    in_maps = build_inputs(x, Wqkv, Wo, log_xi, pi_gate_logit, e_gate_logit)
    nc = build_program()
    nc.finalize()
    res = run_bass_kernel_spmd(nc, in_maps, list(range(8))).results
    out = np.zeros((BATCH, SEQ, D_MODEL), np.float32)
    for core in range(8):
        out[core // 4] += np.asarray(res[core]["out"]).astype(np.float32)
    return out
